# revision 1
# baseline (speedup 1.0000x reference)
"""Trainium2 Bass kernel for nn_Attention_9242769622327.

Math: the reference computes
    qkv = x @ W1.T ; q,k,v = split(qkv)
    score = softmax(k^T v / 4, axis=-1)            # rows sum to 1
    attn  = softmax(einsum('bhnk,bhkc->bhnk', q/4, score), axis=-1)
          = softmax(q/4 * sum_c score)             # sum_c score == 1
          = softmax(q/4)                           # k/v are mathematically dead
    out   = attn @ W2.T
so only the q-projection (first E rows of W1), a per-head (64-wide) softmax,
and the output projection are needed.

Distribution: pure data-parallel over the 32768 = B*S rows; each of the 8
cores handles 4096 rows with the full (transposed, fp16) weights. No
collectives.  fp16 runs the PE at the same 1 cycle/row as bf16 but with a
10-bit mantissa (rel err ~4.5e-4 vs ~3.6e-3 for bf16).

On-chip layout is fully transposed (features on partitions, rows on the free
dim) so no on-chip transposes are needed anywhere:
    qT[n,m]  = sum_k W1qT[k,n] * xT[k,m]          (PE, fp16)
    u        = exp(qT/4)                          (ACT, PSUM->SBUF fp16)
    s[g,m]   = sum_{n in head g} u[n,m]           (PE matmul w/ 0/1 selector)
    rcp      = 1/s                                (DVE reciprocal_approx_fast;
                                                   no Ln -> one ACT table set)
    rb[n,m]  = rcp[head(n),m]                     (PE matmul w/ selector^T,
                                                   K padded to 128 so LDW overlaps)
    aT       = u * rb                             (DVE)
    outT[j,m]= sum_n W2T[n,j] * aT[n,m]           (PE, fp16)

Stripes are software-pipelined: stripe ms runs [64 mm1][8 rb(ms-1)][8 sel]
[64 mm2(ms-1)] as contiguous same-shape matmul blocks on the PE (back-to-back
216ns issue at N=512), with exp/reciprocal/normalize hidden underneath.
Measured: 280.8us on 8 cores, rel err 4.5e-4 (vs ~249us pure-matmul floor).
"""

import sys

sys.path.insert(0, "/opt/trn_rl_repo")

import numpy as np
import ml_dtypes

import concourse.bass as bass
import concourse.bacc as bacc
import concourse.tile as tile
from concourse import mybir
from concourse.bass_utils import run_bass_kernel_spmd

BF16 = mybir.dt.float16  # fp16: same PE rate as bf16, 10-bit mantissa
F32 = mybir.dt.float32
AF = mybir.ActivationFunctionType

N_CORES = 8
B, S, E = 4, 8192, 1024
HEADS, HEAD_DIM = 16, 64
M_TOTAL = B * S                # 32768
M_CORE = M_TOTAL // N_CORES    # 4096 rows per core
MS = 512                       # m-stripe width (moving free dim / PSUM bank)
N_STRIPES = M_CORE // MS       # 8
KC = E // 128                  # 8 contraction chunks
NC_ = E // 128                 # 8 feature chunks

_BF = np.float16


def build_nc() -> bass.Bass:
    nc = bacc.Bacc("TRN2", debug=False)

    xt = nc.dram_tensor("xt", [E, M_CORE], BF16, kind="ExternalInput")
    w1t = nc.dram_tensor("w1t", [E, E], BF16, kind="ExternalInput")
    w2t = nc.dram_tensor("w2t", [E, E], BF16, kind="ExternalInput")
    sel = nc.dram_tensor("sel", [128, NC_ * HEADS], BF16, kind="ExternalInput")
    selt = nc.dram_tensor("selt", [128, NC_ * 128], BF16, kind="ExternalInput")
    outT = nc.dram_tensor("outT", [E, M_CORE], BF16, kind="ExternalOutput")

    xt_v = xt[:, :].rearrange("(c p) m -> p c m", p=128)    # [128, 8, M_CORE]
    w1_v = w1t[:, :].rearrange("(c p) n -> p c n", p=128)   # [128, 8, 1024]
    w2_v = w2t[:, :].rearrange("(c p) j -> p c j", p=128)   # [128, 8, 1024]

    with tile.TileContext(nc) as tc:
        with (
            tc.tile_pool(name="weights", bufs=1) as wpool,
            tc.tile_pool(name="xt", bufs=N_STRIPES) as xpool,
            tc.tile_pool(name="u", bufs=16) as upool,
            tc.tile_pool(name="at", bufs=16) as apool,
            tc.tile_pool(name="small", bufs=3) as spool,
            tc.tile_pool(name="ostage", bufs=8) as opool,
            tc.tile_pool(name="ps_q", bufs=2, space="PSUM") as psq,
            tc.tile_pool(name="ps_s", bufs=2, space="PSUM") as pss,
            tc.tile_pool(name="ps_rb", bufs=2, space="PSUM") as psrb,
            tc.tile_pool(name="ps_o", bufs=2, space="PSUM") as pso,
        ):
            # Per-chunk weight tiles so the first matmuls only wait on the
            # chunks they read, not the whole 4MB of weights.  Load order:
            # w1 + sel (needed by stripe 0's mm1/sel), stripe-0 x chunks,
            # then w2 + selt (not needed until ~18us in).
            # Warm the PE's HAM clock gate with throwaway matmuls on memset
            # scratch while the first weight/x DMAs are in flight, so the
            # first real matmuls run at 2.4 GHz instead of 1.2.
            warm_sb = wpool.tile([128, MS], BF16, name="warm_sb")
            nc.gpsimd.memset(warm_sb[:], 0.0)
            warm_ps = psq.tile([128, MS], F32, tag="q", name="warm_ps")
            for _ in range(16):
                nc.tensor.matmul(
                    warm_ps[:], warm_sb[:, 0:128], warm_sb[:], start=True, stop=True
                )

            w1_k = []
            xt0 = []
            for kc in range(KC):
                t = wpool.tile([128, E], BF16, tag=f"w1_{kc}", name=f"w1k{kc}")
                nc.sync.dma_start(t[:], w1_v[:, kc, :])
                w1_k.append(t)
                tx = xpool.tile([128, MS], BF16, tag=f"xt_{kc}", name=f"xt0_{kc}")
                nc.sync.dma_start(tx[:], xt_v[:, kc, 0:MS])
                xt0.append(tx)
            sel_t = wpool.tile([128, NC_, HEADS], BF16, name="sel_t")
            nc.sync.dma_start(sel_t[:], sel[:, :].rearrange("p (c g) -> p c g", g=HEADS))

            w2_k = []
            for ci in range(NC_):
                t = wpool.tile([128, E], BF16, tag=f"w2_{ci}", name=f"w2k{ci}")
                nc.sync.dma_start(t[:], w2_v[:, ci, :])
                w2_k.append(t)
            selt_t = wpool.tile([128, NC_, 128], BF16, name="selt_t")
            nc.sync.dma_start(selt_t[:], selt[:, :].rearrange("p (c q) -> p c q", q=128))

            # Software pipeline over stripes: while stripe ms runs its
            # q-projection (mm1) + exp + head-sum on the PE, stripe ms-1's
            # normalization (rb broadcast matmul + DVE mul) and output
            # projection (mm2) are interleaved so the PE never waits on the
            # softmax chain.
            prev_u = None       # u tiles of stripe ms-1
            prev_rcp = None     # reciprocal head-sums of stripe ms-1 (bf16)
            prev_ms = -1

            def emit_norm(pu, prcp):
                """rb broadcast matmuls (PE, contiguous block, K padded to 128
                so LDWEIGHTS overlaps like the main GEMM blocks) + DVE muls."""
                ats = []
                for ci in range(NC_):
                    rb_ps = psrb.tile([128, MS], F32, tag="rb", name="rb_ps")
                    nc.tensor.matmul(
                        rb_ps[:], selt_t[:, ci, :], prcp[:], start=True, stop=True
                    )
                    at_t = apool.tile([128, MS], BF16, tag="at", name="at_t")
                    nc.vector.tensor_mul(at_t[:], pu[ci][:], rb_ps[:])
                    ats.append(at_t)
                return ats

            def emit_tail(at_list, ms):
                """Emit mm2 + store for a finished stripe (at tiles ready)."""
                for j in range(NC_):
                    o_ps = pso.tile([128, MS], F32, tag="o", name="o_ps")
                    for ci in range(NC_):
                        nc.tensor.matmul(
                            o_ps[:],
                            w2_k[ci][:, j * 128:(j + 1) * 128],
                            at_list[ci][:],
                            start=(ci == 0),
                            stop=(ci == NC_ - 1),
                        )
                    o_t = opool.tile([128, MS], BF16, tag="ost", name="o_t")
                    nc.scalar.copy(o_t[:], o_ps[:])
                    nc.sync.dma_start(
                        outT[j * 128:(j + 1) * 128, ms * MS:(ms + 1) * MS], o_t[:]
                    )

            for ms in range(N_STRIPES):
                if ms == 0:
                    xt_k = xt0
                else:
                    xt_k = []
                    for kc in range(KC):
                        t = xpool.tile(
                            [128, MS], BF16, tag=f"xt_{kc}", name=f"xt{ms}_{kc}"
                        )
                        nc.sync.dma_start(
                            t[:], xt_v[:, kc, ms * MS:(ms + 1) * MS]
                        )
                        xt_k.append(t)

                # ---- mm1: q-projection, contiguous 64-MM block on PE ----
                u_tiles = []
                q_list = []
                for ci in range(NC_):
                    q_ps = psq.tile([128, MS], F32, tag="q", name="q_ps")
                    for kc in range(KC):
                        nc.tensor.matmul(
                            q_ps[:],
                            w1_k[kc][:, ci * 128:(ci + 1) * 128],
                            xt_k[kc][:],
                            start=(kc == 0),
                            stop=(kc == KC - 1),
                        )
                    u_t = upool.tile([128, MS], BF16, tag="u", name="u_t")
                    nc.scalar.activation(u_t[:], q_ps[:], AF.Exp, scale=0.25)
                    u_tiles.append(u_t)

                # ---- stripe ms-1 normalization (hides exp latency) ----
                at_tiles = emit_norm(prev_u, prev_rcp) if prev_rcp is not None else None

                # ---- head sums (contiguous 8-MM block) + reciprocal ----
                s_ps = pss.tile([HEADS, MS], F32, tag="s", name="s_ps")
                for ci in range(NC_):
                    nc.tensor.matmul(
                        s_ps[:],
                        sel_t[:, ci, :],
                        u_tiles[ci][:],
                        start=(ci == 0),
                        stop=(ci == NC_ - 1),
                    )
                rcp32 = spool.tile([HEADS, MS], F32, tag="rcp32", name="rcp32")
                nc.vector.reciprocal_approx_fast(rcp32[:], s_ps[:])
                # rcp padded to 128 partitions (rows 16+ zeroed on the idle
                # GpSimd engine) so the rb matmul runs with K=128
                rcp_t = spool.tile([128, MS], BF16, tag="rcp", name="rcp_t")
                nc.gpsimd.memset(rcp_t[:], 0.0)
                nc.scalar.copy(rcp_t[0:HEADS, :], rcp32[:])

                # ---- stripe ms-1 output projection ----
                if at_tiles is not None:
                    emit_tail(at_tiles, prev_ms)
                prev_u, prev_rcp, prev_ms = u_tiles, rcp_t, ms

            # epilogue: last stripe's normalization + output projection
            at_tiles = emit_norm(prev_u, prev_rcp)
            emit_tail(at_tiles, prev_ms)
    nc.compile()
    return nc


_NC_CACHE = None
LAST_RESULT = None


def _ensure_ntff_hook():
    """bass_utils' axon trace path needs antenv.axon_hooks, which this
    container's antenv lacks. Provide it + register the ctypes NTFF hook."""
    import types

    try:
        from antenv.axon_hooks import get_axon_ntff_profile_hook  # noqa: F401
        return True
    except ImportError:
        pass
    try:
        import antenv
        from trn_agent_boot.trn_boot import _ntff_profile_via_ctypes

        m = types.ModuleType("antenv.axon_hooks")
        state = {"hook": None}
        m.set_axon_ntff_profile_hook = lambda h: state.__setitem__("hook", h)
        m.get_axon_ntff_profile_hook = lambda: state["hook"]
        sys.modules["antenv.axon_hooks"] = m
        antenv.axon_hooks = m
        m.set_axon_ntff_profile_hook(
            _ntff_profile_via_ctypes("/opt/axon/libaxon_pjrt.so")
        )
        return True
    except Exception as e:  # pragma: no cover
        print(f"ntff hook injection failed: {e}")
        return False


def _selectors():
    # head index of global feature n is n // 64; chunk ci covers n in
    # [128ci, 128ci+128) -> heads 2ci (partitions 0..63) and 2ci+1 (64..127)
    sel = np.zeros((128, NC_, HEADS), np.float32)
    selt = np.zeros((128, NC_, 128), np.float32)  # K padded to 128, rows 16+ zero
    for ci in range(NC_):
        sel[:64, ci, 2 * ci] = 1.0
        sel[64:, ci, 2 * ci + 1] = 1.0
        selt[2 * ci, ci, :64] = 1.0
        selt[2 * ci + 1, ci, 64:] = 1.0
    return (
        np.ascontiguousarray(sel.reshape(128, NC_ * HEADS)).astype(_BF),
        np.ascontiguousarray(selt.reshape(128, NC_ * 128)).astype(_BF),
    )


def kernel(x, W1, W2, heads, trace=False):
    global _NC_CACHE, LAST_RESULT
    x = np.asarray(x, dtype=np.float32)
    W1 = np.asarray(W1, dtype=np.float32)
    W2 = np.asarray(W2, dtype=np.float32)

    X = x.reshape(M_TOTAL, E)
    Xbf = X.astype(_BF)
    XbfT = Xbf.T  # [E, M_TOTAL] view
    w1t = np.ascontiguousarray(W1[:E, :].T).astype(_BF)   # [k, n] = W1q[n, k]
    w2t = np.ascontiguousarray(W2.T).astype(_BF)          # [n, j] = W2[j, n]
    sel, selt = _selectors()

    in_maps = []
    for c in range(N_CORES):
        xt_c = np.ascontiguousarray(XbfT[:, c * M_CORE:(c + 1) * M_CORE])
        in_maps.append(
            {"xt": xt_c, "w1t": w1t, "w2t": w2t, "sel": sel, "selt": selt}
        )

    if _NC_CACHE is None:
        _NC_CACHE = build_nc()

    if trace:
        trace = _ensure_ntff_hook()

    res = run_bass_kernel_spmd(_NC_CACHE, in_maps, list(range(N_CORES)), trace=trace)
    LAST_RESULT = res

    OT = np.concatenate(
        [np.asarray(res.results[c]["outT"]).astype(np.float32) for c in range(N_CORES)],
        axis=1,
    )
    return np.ascontiguousarray(OT.T).reshape(B, S, E)



# revision 2
# speedup vs baseline: 1.3833x; 1.3833x over previous
"""Trainium2 Bass kernel for nn_Attention_9242769622327.

Math: the reference computes
    qkv = x @ W1.T ; q,k,v = split(qkv)
    score = softmax(k^T v / 4, axis=-1)            # rows sum to 1
    attn  = softmax(einsum('bhnk,bhkc->bhnk', q/4, score), axis=-1)
          = softmax(q/4)                           # k/v are mathematically dead
    out   = attn @ W2.T
so only the q-projection (first E rows of W1), a per-head (64-wide) softmax,
and the output projection are needed.

Distribution: pure data-parallel over the 32768 = B*S rows; each of the 8
cores handles 4096 rows with the full (transposed) weights. No collectives.

Both big GEMMs run in fp8 e4m3 with MatmulPerfMode.DoubleRow (two K=128
reduction tiles per instruction -> 2x PE throughput).  fp8's ~2.7% per-value
quantization noise is tamed by centering:
  - mm1 (q-projection): the softmax's /4 temperature + normalization damp the
    q error ~4x, so plain fp8 x / fp8 (32*W1q) is fine (~0.9% final).
  - mm2: attn rows sum to exactly 1 per 64-wide head, so feed the PE
    at0 = A*(u - 1)*rb   (u = exp(q/4), rb = broadcast 64/s, A=16)
    whose magnitude is ~4x smaller than A*u*rb, and add the exact
    complement with a tiny K=16 bf16 matmul per output chunk:
        o_ps[j,m] = sum_h rcp[h,m] * w2sum[h,j] + sum_n W2q[n,j] * at0[n,m]
                  = A*64*32*outT[j,m]
    where w2sum[h,j] = sum_{n in head h} 32*W2T[n,j] is an exact fp32
    per-head column sum (so the dominant "mean attention" term carries no
    fp8 weight error at all).  Host-emulated rel err: 1.3e-2 (gate 2e-2).

On-chip layout is fully transposed (features on partitions, rows on the free
dim) so no on-chip transposes are needed anywhere:
    qT[n,m]  = sum_k W1qT[k,n] * xT[k,m]          (PE, fp8 DoubleRow)
    u        = exp(qT/128)                        (ACT, PSUM->SBUF fp16)
    s[g,m]   = sum_{n in head g} u[n,m]/64        (PE matmul w/ 1/64 selector)
    rcp      = 16*64/s                            (DVE reciprocal + ACT copy)
    rb[n,m]  = rcp[head(n),m]                     (PE matmul w/ selector^T)
    at0      = (u - 1) * rb                       (DVE scalar_tensor_tensor,
                                                   fp8 out)
    outT[j,m]= corr(j) + sum_n W2T[n,j]*at0[n,m]  (PE, fp8 DoubleRow + K=16
                                                   bf16 correction)

Per stripe the PE runs 88 matmuls (~16us): [2 rb][8x(4 DR mm1 + rb + sel)]
[2 sel][8x(corr + 4 DR mm2)], with rb/sel interleaved into the mm1 loop so
the PE never waits on the exp (ACT) or at0 (DVE) chains.
"""

import sys

sys.path.insert(0, "/opt/trn_rl_repo")

import numpy as np
import ml_dtypes

import concourse.bass as bass
import concourse.bacc as bacc
import concourse.tile as tile
from concourse import mybir
from concourse.bass_utils import run_bass_kernel_spmd

F16 = mybir.dt.float16
FP8 = mybir.dt.float8e4
F32 = mybir.dt.float32
AF = mybir.ActivationFunctionType
ALU = mybir.AluOpType
DR = mybir.MatmulPerfMode.DoubleRow

N_CORES = 8
B, S, E = 4, 8192, 1024
HEADS, HEAD_DIM = 16, 64
M_TOTAL = B * S                # 32768
M_CORE = M_TOTAL // N_CORES    # 4096 rows per core
MS = 512                       # m-stripe width (moving free dim / PSUM bank)
KC2 = E // 256                 # 4 DoubleRow contraction pairs
NC_ = E // 128                 # 8 feature chunks
A_SCALE = 16.0                 # fp8 scale for the centered attn
W_SCALE = 32.0                 # host pre-scale of W1q/W2 (std 1/32 -> ~1)
OUT_DESCALE = 1.0 / (A_SCALE * 64.0 * W_SCALE)

_E4 = ml_dtypes.float8_e4m3
_F16 = np.float16


def build_nc(m_core=M_CORE) -> bass.Bass:
    n_stripes = m_core // MS
    nc = bacc.Bacc("TRN2", debug=False)

    xt = nc.dram_tensor("xt", [E, m_core], FP8, kind="ExternalInput")
    w1t = nc.dram_tensor("w1t", [E, E], FP8, kind="ExternalInput")
    w2t = nc.dram_tensor("w2t", [E, E], FP8, kind="ExternalInput")
    sel = nc.dram_tensor("sel", [128, NC_ * HEADS], F16, kind="ExternalInput")
    selt = nc.dram_tensor("selt", [128, NC_ * 128], F16, kind="ExternalInput")
    w2s = nc.dram_tensor("w2s", [HEADS, E], F16, kind="ExternalInput")
    outT = nc.dram_tensor("outT", [E, m_core], F16, kind="ExternalOutput")

    xt_v = xt[:, :].rearrange("(c p) m -> p c m", p=128)    # [128, 8, m_core]
    w1_v = w1t[:, :].rearrange("(c p) n -> p c n", p=128)   # [128, 8, 1024]
    w2_v = w2t[:, :].rearrange("(c p) j -> p c j", p=128)   # [128, 8, 1024]

    with tile.TileContext(nc) as tc:
        with (
            tc.tile_pool(name="weights", bufs=1) as wpool,
            tc.tile_pool(name="xt", bufs=n_stripes) as xpool,
            tc.tile_pool(name="u", bufs=16) as upool,
            tc.tile_pool(name="at", bufs=8) as apool,
            tc.tile_pool(name="small", bufs=2) as spool,
            tc.tile_pool(name="ostage", bufs=8) as opool,
            tc.tile_pool(name="ps_q", bufs=3, space="PSUM") as psq,
            tc.tile_pool(name="ps_s", bufs=1, space="PSUM") as pss,
            tc.tile_pool(name="ps_rb", bufs=2, space="PSUM") as psrb,
            tc.tile_pool(name="ps_o", bufs=2, space="PSUM") as pso,
        ):
            # Warm the PE's HAM clock gate with throwaway matmuls on memset
            # scratch while the first weight/x DMAs are in flight.
            warm_sb = wpool.tile([128, MS], F16, name="warm_sb")
            nc.gpsimd.memset(warm_sb[:], 0.0)
            warm_ps = psq.tile([128, MS], F32, tag="q", name="warm_ps")
            for _ in range(16):
                nc.tensor.matmul(
                    warm_ps[:], warm_sb[:, 0:128], warm_sb[:], start=True, stop=True
                )

            # Weight/selector loads, ordered so stripe 0's mm1 can start
            # as early as possible: w1 pair 0 + x0 pair 0 first.
            w1_k = []   # 4 tiles [128, 2, E] fp8 (DoubleRow pairs)
            x_k = [[None] * KC2 for _ in range(n_stripes)]
            for kc2 in range(KC2):
                t = wpool.tile([128, 2, E], FP8, name=f"w1k{kc2}")
                nc.sync.dma_start(t[:], w1_v[:, 2 * kc2:2 * kc2 + 2, :])
                w1_k.append(t)
                tx = xpool.tile([128, 2, MS], FP8, tag=f"xt_{kc2}", name=f"xt0_{kc2}")
                nc.sync.dma_start(tx[:], xt_v[:, 2 * kc2:2 * kc2 + 2, 0:MS])
                x_k[0][kc2] = tx
            sel_t = wpool.tile([128, NC_, HEADS], F16, name="sel_t")
            nc.sync.dma_start(sel_t[:], sel[:, :].rearrange("p (c g) -> p c g", g=HEADS))
            selt_t = wpool.tile([128, NC_, 128], F16, name="selt_t")
            nc.sync.dma_start(selt_t[:], selt[:, :].rearrange("p (c q) -> p c q", q=128))
            w2s_t = wpool.tile([HEADS, E], F16, name="w2s_t")
            nc.sync.dma_start(w2s_t[:], w2s[:, :])

            w2_k = []   # 4 tiles [128, 2, E] fp8 (pairs of n-chunks)
            for c2 in range(KC2):
                t = wpool.tile([128, 2, E], FP8, name=f"w2k{c2}")
                nc.sync.dma_start(t[:], w2_v[:, 2 * c2:2 * c2 + 2, :])
                w2_k.append(t)

            def prefetch_x(ms):
                if ms < 1 or ms >= n_stripes:
                    return
                for kc2 in range(KC2):
                    tx = xpool.tile(
                        [128, 2, MS], FP8, tag=f"xt_{kc2}", name=f"xt{ms}_{kc2}"
                    )
                    nc.sync.dma_start(
                        tx[:], xt_v[:, 2 * kc2:2 * kc2 + 2, ms * MS:(ms + 1) * MS]
                    )
                    x_k[ms][kc2] = tx
            prefetch_x(1)

            # rcp tiles: rows 16.. must stay zero (rb matmul runs K=128 so
            # its LDWEIGHTS overlaps like the big GEMMs); zero them once and
            # alternate between two fixed buffers.
            rcp_bufs = []
            for i in range(2):
                t = wpool.tile([128, MS], F16, name=f"rcp{i}")
                nc.gpsimd.memset(t[:], 0.0)
                rcp_bufs.append(t)

            def emit_rb(prev, ci):
                """One rb broadcast matmul + the fused (u-1)*rb DVE op that
                quantizes the centered attn to fp8."""
                rb_ps = psrb.tile([128, MS], F32, tag="rb", name="rb_ps")
                nc.tensor.matmul(
                    rb_ps[:], selt_t[:, ci, :], prev["rcp"][:], start=True, stop=True
                )
                pair, half = divmod(ci, 2)
                nc.vector.scalar_tensor_tensor(
                    prev["at"][pair][:, half, :],
                    prev["u"][ci][:],
                    -1.0,
                    rb_ps[:],
                    op0=ALU.add,
                    op1=ALU.mult,
                )

            def emit_mm2(prev):
                """8 j-groups: [K=16 bf16 correction, 4 fp8 DR matmuls]."""
                for j in range(NC_):
                    o_ps = pso.tile([128, MS], F32, tag="o", name="o_ps")
                    nc.tensor.matmul(
                        o_ps[:],
                        w2s_t[:, j * 128:(j + 1) * 128],
                        prev["rcp"][0:HEADS, :],
                        start=True,
                        stop=False,
                        skip_group_check=True,
                    )
                    for c2 in range(KC2):
                        nc.tensor.matmul(
                            o_ps[:],
                            w2_k[c2][:, :, j * 128:(j + 1) * 128],
                            prev["at"][c2][:],
                            start=False,
                            stop=(c2 == KC2 - 1),
                            perf_mode=DR,
                            skip_group_check=True,
                        )
                    o_t = opool.tile([128, MS], F16, tag="ost", name="o_t")
                    nc.scalar.mul(o_t[:], o_ps[:], OUT_DESCALE)
                    nc.sync.dma_start(
                        outT[j * 128:(j + 1) * 128,
                             prev["ms"] * MS:(prev["ms"] + 1) * MS],
                        o_t[:],
                    )

            prev = None
            for ms in range(n_stripes):
                prefetch_x(ms + 1)
                cur = {
                    "ms": ms,
                    "u": [],
                    "at": [
                        apool.tile([128, 2, MS], FP8, tag=f"at{p}", name=f"at{ms}_{p}")
                        for p in range(KC2)
                    ],
                    "rcp": rcp_bufs[ms % 2],
                }

                # Head of block: 2 rb matmuls of the previous stripe (the
                # rest interleave into the mm1 loop so the PE stays ahead of
                # the DVE's 823ns/op at0 chain).
                if prev is not None:
                    emit_rb(prev, 0)
                    emit_rb(prev, 1)

                s_ps = pss.tile([HEADS, MS], F32, tag="s", name="s_ps")
                for ci in range(NC_):
                    q_ps = psq.tile([128, MS], F32, tag="q", name="q_ps")
                    for kc2 in range(KC2):
                        nc.tensor.matmul(
                            q_ps[:],
                            w1_k[kc2][:, :, ci * 128:(ci + 1) * 128],
                            x_k[ms][kc2][:],
                            start=(kc2 == 0),
                            stop=(kc2 == KC2 - 1),
                            perf_mode=DR,
                        )
                    u_t = upool.tile([128, MS], F16, tag="u", name="u_t")
                    nc.scalar.activation(u_t[:], q_ps[:], AF.Exp, scale=1.0 / 128.0)
                    cur["u"].append(u_t)

                    if prev is not None and ci + 2 < NC_:
                        emit_rb(prev, ci + 2)
                    if ci >= 2:
                        nc.tensor.matmul(
                            s_ps[:],
                            sel_t[:, ci - 2, :],
                            cur["u"][ci - 2][:],
                            start=(ci - 2 == 0),
                            stop=False,
                        )
                for ci in (NC_ - 2, NC_ - 1):
                    nc.tensor.matmul(
                        s_ps[:],
                        sel_t[:, ci, :],
                        cur["u"][ci][:],
                        start=False,
                        stop=(ci == NC_ - 1),
                    )

                rcp32 = spool.tile([HEADS, MS], F32, tag="rcp32", name="rcp32")
                nc.vector.reciprocal_approx_fast(rcp32[:], s_ps[:])
                nc.scalar.mul(cur["rcp"][0:HEADS, :], rcp32[:], A_SCALE)

                if prev is not None:
                    emit_mm2(prev)
                prev = cur

            # epilogue: last stripe's rb/at0 + output projection
            for ci in range(NC_):
                emit_rb(prev, ci)
            emit_mm2(prev)
    nc.compile()
    return nc


_NC_CACHE = None
LAST_RESULT = None


def _ensure_ntff_hook():
    """bass_utils' axon trace path needs antenv.axon_hooks, which this
    container's antenv lacks. Provide it + register the ctypes NTFF hook."""
    import types

    try:
        from antenv.axon_hooks import get_axon_ntff_profile_hook  # noqa: F401
        return True
    except ImportError:
        pass
    try:
        import antenv
        from trn_agent_boot.trn_boot import _ntff_profile_via_ctypes

        m = types.ModuleType("antenv.axon_hooks")
        state = {"hook": None}
        m.set_axon_ntff_profile_hook = lambda h: state.__setitem__("hook", h)
        m.get_axon_ntff_profile_hook = lambda: state["hook"]
        sys.modules["antenv.axon_hooks"] = m
        antenv.axon_hooks = m
        m.set_axon_ntff_profile_hook(
            _ntff_profile_via_ctypes("/opt/axon/libaxon_pjrt.so")
        )
        return True
    except Exception as e:  # pragma: no cover
        print(f"ntff hook injection failed: {e}")
        return False


def _selectors():
    # head index of global feature n is n // 64; chunk ci covers n in
    # [128ci, 128ci+128) -> heads 2ci (partitions 0..63) and 2ci+1 (64..127)
    sel = np.zeros((128, NC_, HEADS), np.float32)
    selt = np.zeros((128, NC_, 128), np.float32)  # K padded to 128, rows 16+ zero
    for ci in range(NC_):
        sel[:64, ci, 2 * ci] = 1.0 / 64.0
        sel[64:, ci, 2 * ci + 1] = 1.0 / 64.0
        selt[2 * ci, ci, :64] = 1.0
        selt[2 * ci + 1, ci, 64:] = 1.0
    return (
        np.ascontiguousarray(sel.reshape(128, NC_ * HEADS)).astype(_F16),
        np.ascontiguousarray(selt.reshape(128, NC_ * 128)).astype(_F16),
    )


def _prep_weights(W1, W2):
    w1t = np.ascontiguousarray(W1[:E, :].T * W_SCALE).astype(_E4)  # [k, n]
    w2t = np.ascontiguousarray(W2.T * W_SCALE).astype(_E4)         # [n, j]
    # exact per-head column sums of 32*W2T (fp32, then fp16)
    w2s = (W2.T * W_SCALE).reshape(HEADS, HEAD_DIM, E).sum(axis=1).astype(_F16)
    return w1t, w2t, w2s


def kernel(x, W1, W2, heads, trace=False):
    global _NC_CACHE, LAST_RESULT
    x = np.asarray(x, dtype=np.float32)
    W1 = np.asarray(W1, dtype=np.float32)
    W2 = np.asarray(W2, dtype=np.float32)

    X = x.reshape(M_TOTAL, E)
    XT8 = np.ascontiguousarray(X.T).astype(_E4)  # [E, M_TOTAL]
    w1t, w2t, w2s = _prep_weights(W1, W2)
    sel, selt = _selectors()

    in_maps = []
    for c in range(N_CORES):
        xt_c = np.ascontiguousarray(XT8[:, c * M_CORE:(c + 1) * M_CORE])
        in_maps.append(
            {"xt": xt_c, "w1t": w1t, "w2t": w2t, "sel": sel, "selt": selt,
             "w2s": w2s}
        )

    if _NC_CACHE is None:
        _NC_CACHE = build_nc()

    if trace:
        trace = _ensure_ntff_hook()

    res = run_bass_kernel_spmd(_NC_CACHE, in_maps, list(range(N_CORES)), trace=trace)
    LAST_RESULT = res

    OT = np.concatenate(
        [np.asarray(res.results[c]["outT"]).astype(np.float32) for c in range(N_CORES)],
        axis=1,
    )
    return np.ascontiguousarray(OT.T).reshape(B, S, E)


# revision 3
# speedup vs baseline: 1.5908x; 1.1500x over previous
"""Trainium2 Bass kernel for nn_Attention_9242769622327.

Math: the reference computes
    qkv = x @ W1.T ; q,k,v = split(qkv)
    score = softmax(k^T v / 4, axis=-1)            # rows sum to 1
    attn  = softmax(einsum('bhnk,bhkc->bhnk', q/4, score), axis=-1)
          = softmax(q/4)                           # k/v are mathematically dead
    out   = attn @ W2.T
so only the q-projection (first E rows of W1), a per-head (64-wide) softmax,
and the output projection are needed.

Distribution: pure data-parallel over the 32768 = B*S rows; each of the 8
cores handles 4096 rows with the full (transposed) weights. No collectives.

EVERY matmul runs as fp8 e4m3 MatmulPerfMode.DoubleRow (two K-tiles per
instruction, 2x PE throughput; measured 216ns per [*,2,*]x[*,2,512]
instruction back-to-back — and mixing any f16 matmul into the fp8 stream
was measured to stall the PE at ~2x cycle time for ~us, so the PE stream
is kept mode-pure).  fp8's ~2.7% per-value quantization noise is tamed by
centering every fp8-quantized quantity around its known mean:
  - mm1 (q-projection): softmax's /4 temperature + normalization damp the
    error ~4x; plain fp8 x / fp8 (32*W1q) gives ~0.9% final.
  - attn: rows sum to exactly 1 per 64-wide head, so the PE gets
    at0 = (u - 1)*rb   (u = exp(q/4) f16, rb = A*64/s broadcast, A=16)
    which is ~4x smaller than u*rb.
  - rcp: rcpc = A*64/s - A (+-2.5) is quantized fp8; the exact constant A
    is restored through a second K-tile whose selector row multiplies a
    constant-A row, so rb = selt0^T@rcpc + A exactly in fp32 PSUM.
  - mm2 constant part: sum_h rcp*w2sum splits into the exact f32 bias
    colsum(32*W2T)/2048 (applied in the output-copy ACT) plus the tiny
    centered fp8 matmul rcpc @ w2sum8.
  - head sums: s comes from an fp8 copy of u (DVE); the 2.7%/sqrt(64)
    coherent error this adds is ~0.3%.
Host-emulated + CoreSim rel err: 1.35e-2 (gate 2e-2).

On-chip layout is fully transposed (features on partitions, rows on the
free dim) so no on-chip transposes are needed anywhere:
    qT[n,m]  = sum_k W1qT[k,n]*xT[k,m]        (PE DR, fp8, PSUM=32q)
    u        = exp(qT/128)                    (ACT, PSUM->SBUF f16)
    u8       = fp8(u)                         (DVE copy, pair tiles)
    s[g,m]/64= sum_{n in g} u8[n,m]/64        (PE DR w/ 1/64 selector pairs)
    rcpc     = fp8(A*64/s - A)                (DVE reciprocal + tensor_scalar)
    rb[n,m]  = rcpc[head(n),m] + A            (PE DR w/ selector^T + const row)
    at0      = (u - 1)*rb                     (DVE scalar_tensor_tensor, fp8)
    outT[j,m]= [rcpc@w2sum8 + sum_n W2T[n,j]*at0[n,m]]/(A*64*32) + bias_j
                                              (PE DR; ACT Identity w/ bias)

Per stripe the PE issues 84 uniform DR matmuls: 8 rb (interleaved into the
mm1 loop so the PE stays ahead of the DVE's at0 chain), 32 mm1, 40 mm2
(8 j-groups of [rcpc-corr + 4 at0]), 4 sel — ~218ns each sustained.
"""

import sys

sys.path.insert(0, "/opt/trn_rl_repo")

import numpy as np
import ml_dtypes

import concourse.bass as bass
import concourse.bacc as bacc
import concourse.tile as tile
from concourse import mybir
from concourse.bass_utils import run_bass_kernel_spmd

F16 = mybir.dt.float16
FP8 = mybir.dt.float8e4
F32 = mybir.dt.float32
AF = mybir.ActivationFunctionType
ALU = mybir.AluOpType
DR = mybir.MatmulPerfMode.DoubleRow

N_CORES = 8
B, S, E = 4, 8192, 1024
HEADS, HEAD_DIM = 16, 64
M_TOTAL = B * S                # 32768
M_CORE = M_TOTAL // N_CORES    # 4096 rows per core
MS = 512                       # m-stripe width (moving free dim / PSUM bank)
KC2 = E // 256                 # 4 DoubleRow contraction pairs
NC_ = E // 128                 # 8 feature chunks
A_SCALE = 16.0                 # fp8 scale for the centered attn
W_SCALE = 32.0                 # host pre-scale of W1q/W2 (std 1/32 -> ~1)
OUT_DESCALE = 1.0 / (A_SCALE * 64.0 * W_SCALE)

_E4 = ml_dtypes.float8_e4m3
_F16 = np.float16


def build_nc(m_core=M_CORE) -> bass.Bass:
    n_stripes = m_core // MS
    nc = bacc.Bacc("TRN2", debug=False)

    xt = nc.dram_tensor("xt", [E, m_core], FP8, kind="ExternalInput")
    w1t = nc.dram_tensor("w1t", [E, E], FP8, kind="ExternalInput")
    w2t = nc.dram_tensor("w2t", [E, E], FP8, kind="ExternalInput")
    sel8 = nc.dram_tensor("sel8", [128, KC2 * 2 * HEADS], FP8, kind="ExternalInput")
    selt8 = nc.dram_tensor("selt8", [HEADS, NC_ * 2 * 128], FP8, kind="ExternalInput")
    w2sum8 = nc.dram_tensor("w2sum8", [HEADS, NC_ * 2 * 128], FP8, kind="ExternalInput")
    biasd = nc.dram_tensor("biasd", [128, NC_], F32, kind="ExternalInput")
    outT = nc.dram_tensor("outT", [E, m_core], F16, kind="ExternalOutput")

    xt_v = xt[:, :].rearrange("(c p) m -> p c m", p=128)    # [128, 8, m_core]
    w1_v = w1t[:, :].rearrange("(c p) n -> p c n", p=128)   # [128, 8, 1024]
    w2_v = w2t[:, :].rearrange("(c p) j -> p c j", p=128)   # [128, 8, 1024]

    with tile.TileContext(nc) as tc:
        with (
            tc.tile_pool(name="weights", bufs=1) as wpool,
            tc.tile_pool(name="xt", bufs=n_stripes) as xpool,
            tc.tile_pool(name="u", bufs=16) as upool,
            tc.tile_pool(name="u8", bufs=4) as u8pool,
            tc.tile_pool(name="at", bufs=8) as apool,
            tc.tile_pool(name="small", bufs=2) as spool,
            tc.tile_pool(name="ostage", bufs=8) as opool,
            tc.tile_pool(name="ps_q", bufs=3, space="PSUM") as psq,
            tc.tile_pool(name="ps_s", bufs=1, space="PSUM") as pss,
            tc.tile_pool(name="ps_rb", bufs=2, space="PSUM") as psrb,
            tc.tile_pool(name="ps_o", bufs=2, space="PSUM") as pso,
        ):
            # Warm the PE's HAM clock gate with throwaway DR matmuls (same
            # mode as the real stream) while the first DMAs are in flight.
            warm_sb = wpool.tile([128, 2, MS], FP8, name="warm_sb")
            nc.gpsimd.memset(warm_sb[:], 0.0)
            warm_ps = psq.tile([128, MS], F32, tag="q", name="warm_ps")
            for _ in range(16):
                nc.tensor.matmul(
                    warm_ps[:], warm_sb[:, :, 0:128], warm_sb[:],
                    start=True, stop=True, perf_mode=DR,
                )

            # Weight/selector loads, ordered so stripe 0's mm1 can start
            # as early as possible.
            w1_k = []   # 4 tiles [128, 2, E] fp8 (DoubleRow pairs)
            x_k = [[None] * KC2 for _ in range(n_stripes)]
            for kc2 in range(KC2):
                t = wpool.tile([128, 2, E], FP8, name=f"w1k{kc2}")
                nc.sync.dma_start(t[:], w1_v[:, 2 * kc2:2 * kc2 + 2, :])
                w1_k.append(t)
                tx = xpool.tile([128, 2, MS], FP8, tag=f"xt_{kc2}", name=f"xt0_{kc2}")
                nc.sync.dma_start(tx[:], xt_v[:, 2 * kc2:2 * kc2 + 2, 0:MS])
                x_k[0][kc2] = tx
            sel8_t = wpool.tile([128, KC2, 2, HEADS], FP8, name="sel8_t")
            nc.sync.dma_start(
                sel8_t[:], sel8[:, :].rearrange("p (c i g) -> p c i g", i=2, g=HEADS)
            )
            selt8_t = wpool.tile([HEADS, NC_, 2, 128], FP8, name="selt8_t")
            nc.sync.dma_start(
                selt8_t[:], selt8[:, :].rearrange("p (c i q) -> p c i q", i=2, q=128)
            )
            w2sum8_t = wpool.tile([HEADS, NC_, 2, 128], FP8, name="w2sum8_t")
            nc.sync.dma_start(
                w2sum8_t[:], w2sum8[:, :].rearrange("p (c i q) -> p c i q", i=2, q=128)
            )
            bias_t = wpool.tile([128, NC_], F32, name="bias_t")
            nc.sync.dma_start(bias_t[:], biasd[:, :])

            w2_k = []   # 4 tiles [128, 2, E] fp8 (pairs of n-chunks)
            for c2 in range(KC2):
                t = wpool.tile([128, 2, E], FP8, name=f"w2k{c2}")
                nc.sync.dma_start(t[:], w2_v[:, 2 * c2:2 * c2 + 2, :])
                w2_k.append(t)

            def prefetch_x(ms):
                if ms < 1 or ms >= n_stripes:
                    return
                for kc2 in range(KC2):
                    tx = xpool.tile(
                        [128, 2, MS], FP8, tag=f"xt_{kc2}", name=f"xt{ms}_{kc2}"
                    )
                    nc.sync.dma_start(
                        tx[:], xt_v[:, 2 * kc2:2 * kc2 + 2, ms * MS:(ms + 1) * MS]
                    )
                    x_k[ms][kc2] = tx
            prefetch_x(1)

            # rcpc tiles [16, 2, MS] fp8: [:,0,:] = centered reciprocal
            # (DVE-written per stripe), [:,1,:] row 0 = A (the decenter
            # constant, written once), rest zero.  Two alternating buffers.
            rcpc_bufs = []
            for i in range(2):
                t = wpool.tile([HEADS, 2, MS], FP8, name=f"rcpc{i}")
                nc.gpsimd.memset(t[:], 0.0)
                nc.gpsimd.memset(t[0:1, 1, :], A_SCALE)
                rcpc_bufs.append(t)

            def emit_rb_at0(prev, ci):
                """rb broadcast DR matmul + fused (u-1)*rb DVE op -> fp8."""
                rb_ps = psrb.tile([128, MS], F32, tag="rb", name="rb_ps")
                nc.tensor.matmul(
                    rb_ps[:], selt8_t[:, ci, :, :], prev["rcpc"][:],
                    start=True, stop=True, perf_mode=DR,
                )
                pair, half = divmod(ci, 2)
                nc.vector.scalar_tensor_tensor(
                    prev["at"][pair][:, half, :],
                    prev["u"][ci][:],
                    -1.0,
                    rb_ps[:],
                    op0=ALU.add,
                    op1=ALU.mult,
                )

            def emit_mm2(prev, js):
                """j-groups: [rcpc-corr DR (K=16), 4 fp8 DR at0 matmuls]."""
                for j in js:
                    o_ps = pso.tile([128, MS], F32, tag="o", name="o_ps")
                    nc.tensor.matmul(
                        o_ps[:],
                        w2sum8_t[:, j, :, :],
                        prev["rcpc"][:],
                        start=True,
                        stop=False,
                        perf_mode=DR,
                        skip_group_check=True,
                    )
                    for c2 in range(KC2):
                        nc.tensor.matmul(
                            o_ps[:],
                            w2_k[c2][:, :, j * 128:(j + 1) * 128],
                            prev["at"][c2][:],
                            start=False,
                            stop=(c2 == KC2 - 1),
                            perf_mode=DR,
                            skip_group_check=True,
                        )
                    o_t = opool.tile([128, MS], F16, tag="ost", name="o_t")
                    nc.scalar.activation(
                        o_t[:], o_ps[:], AF.Identity,
                        bias=bias_t[:, j:j + 1], scale=OUT_DESCALE,
                    )
                    nc.sync.dma_start(
                        outT[j * 128:(j + 1) * 128,
                             prev["ms"] * MS:(prev["ms"] + 1) * MS],
                        o_t[:],
                    )

            def emit_sel(cur):
                """4 DR head-sum matmuls on the fp8 u pairs + rcp chain."""
                s_ps = pss.tile([HEADS, MS], F32, tag="s", name="s_ps")
                for c2 in range(KC2):
                    nc.tensor.matmul(
                        s_ps[:],
                        sel8_t[:, c2, :, :],
                        cur["u8"][c2][:],
                        start=(c2 == 0),
                        stop=(c2 == KC2 - 1),
                        perf_mode=DR,
                    )
                rcp32 = spool.tile([HEADS, MS], F32, tag="rcp32", name="rcp32")
                nc.vector.reciprocal_approx_fast(rcp32[:], s_ps[:])
                nc.vector.tensor_scalar(
                    cur["rcpc"][:, 0, :], rcp32[:], A_SCALE, A_SCALE,
                    op0=ALU.mult, op1=ALU.subtract,
                )

            prev = None
            for ms in range(n_stripes):
                prefetch_x(ms + 1)
                cur = {
                    "ms": ms,
                    "u": [],
                    "u8": [
                        u8pool.tile([128, 2, MS], FP8, tag=f"u8{p}", name=f"u8{ms}_{p}")
                        for p in range(KC2)
                    ],
                    "at": [
                        apool.tile([128, 2, MS], FP8, tag=f"at{p}", name=f"at{ms}_{p}")
                        for p in range(KC2)
                    ],
                    "rcpc": rcpc_bufs[ms % 2],
                }

                # Head of block: 2 rb matmuls of the previous stripe; the
                # rest interleave into the mm1 loop so the PE stays just
                # ahead of the DVE's 823ns/op at0 chain (psrb has 2 bufs).
                if prev is not None:
                    emit_rb_at0(prev, 0)
                    emit_rb_at0(prev, 1)

                for ci in range(NC_):
                    q_ps = psq.tile([128, MS], F32, tag="q", name="q_ps")
                    for kc2 in range(KC2):
                        nc.tensor.matmul(
                            q_ps[:],
                            w1_k[kc2][:, :, ci * 128:(ci + 1) * 128],
                            x_k[ms][kc2][:],
                            start=(kc2 == 0),
                            stop=(kc2 == KC2 - 1),
                            perf_mode=DR,
                        )
                    u_t = upool.tile([128, MS], F16, tag="u", name="u_t")
                    nc.scalar.activation(u_t[:], q_ps[:], AF.Exp, scale=1.0 / 128.0)
                    cur["u"].append(u_t)
                    if prev is not None and ci + 2 < NC_:
                        emit_rb_at0(prev, ci + 2)

                # u8 copies (DVE) emitted after all at0 ops so the at0
                # chain (needed first, by mm2) drains first.
                for ci in range(NC_):
                    nc.vector.tensor_copy(
                        cur["u8"][ci // 2][:, ci % 2, :], cur["u"][ci][:]
                    )

                if prev is not None:
                    emit_mm2(prev, range(6))
                emit_sel(cur)
                if prev is not None:
                    emit_mm2(prev, range(6, NC_))
                prev = cur

            # epilogue: last stripe's rb/at0 + output projection
            for ci in range(NC_):
                emit_rb_at0(prev, ci)
            emit_mm2(prev, range(NC_))
    nc.compile()
    return nc


_NC_CACHE = None
LAST_RESULT = None


def _ensure_ntff_hook():
    """bass_utils' axon trace path needs antenv.axon_hooks, which this
    container's antenv lacks. Provide it + register the ctypes NTFF hook."""
    import types

    try:
        from antenv.axon_hooks import get_axon_ntff_profile_hook  # noqa: F401
        return True
    except ImportError:
        pass
    try:
        import antenv
        from trn_agent_boot.trn_boot import _ntff_profile_via_ctypes

        m = types.ModuleType("antenv.axon_hooks")
        state = {"hook": None}
        m.set_axon_ntff_profile_hook = lambda h: state.__setitem__("hook", h)
        m.get_axon_ntff_profile_hook = lambda: state["hook"]
        sys.modules["antenv.axon_hooks"] = m
        antenv.axon_hooks = m
        m.set_axon_ntff_profile_hook(
            _ntff_profile_via_ctypes("/opt/axon/libaxon_pjrt.so")
        )
        return True
    except Exception as e:  # pragma: no cover
        print(f"ntff hook injection failed: {e}")
        return False


def _selectors():
    # head index of global feature n is n // 64; chunk ci covers n in
    # [128ci, 128ci+128) -> heads 2ci (partitions 0..63), 2ci+1 (64..127)
    sel8 = np.zeros((128, KC2, 2, HEADS), np.float32)
    for c2 in range(KC2):
        for i in range(2):
            ci = 2 * c2 + i
            sel8[:64, c2, i, 2 * ci] = 1.0 / 64.0
            sel8[64:, c2, i, 2 * ci + 1] = 1.0 / 64.0
    # selt8[:, ci, 0, :]: 0/1 head selector; [:, ci, 1, :]: row 0 = 1.0,
    # which multiplies the constant-A row of rcpc_pad -> rb = rcpc + A.
    selt8 = np.zeros((HEADS, NC_, 2, 128), np.float32)
    for ci in range(NC_):
        selt8[2 * ci, ci, 0, :64] = 1.0
        selt8[2 * ci + 1, ci, 0, 64:] = 1.0
        selt8[0, ci, 1, :] = 1.0
    return (
        np.ascontiguousarray(sel8.reshape(128, KC2 * 2 * HEADS)).astype(_E4),
        np.ascontiguousarray(selt8.reshape(HEADS, NC_ * 2 * 128)).astype(_E4),
    )


def _prep_weights(W1, W2):
    w1t = np.ascontiguousarray(W1[:E, :].T * W_SCALE).astype(_E4)  # [k, n]
    w2t = np.ascontiguousarray(W2.T * W_SCALE).astype(_E4)         # [n, j]
    # exact per-head column sums of 32*W2T (fp32)
    w2sum32 = (W2.T * W_SCALE).reshape(HEADS, HEAD_DIM, E).sum(axis=1)
    # w2sum8[:, j, 0, :] = fp8 per-head colsums (multiplies rcpc);
    # [:, j, 1, :] = 0 (the exact constant part goes through bias).
    w2sum8 = np.zeros((HEADS, NC_, 2, 128), np.float32)
    for j in range(NC_):
        w2sum8[:, j, 0, :] = w2sum32[:, j * 128:(j + 1) * 128]
    bias = (w2sum32.sum(axis=0) * A_SCALE * OUT_DESCALE).astype(np.float32)
    biasd = np.ascontiguousarray(bias.reshape(NC_, 128).T)         # [128, NC_]
    return (
        w1t,
        w2t,
        np.ascontiguousarray(w2sum8.reshape(HEADS, NC_ * 2 * 128)).astype(_E4),
        biasd,
    )


def kernel(x, W1, W2, heads, trace=False):
    global _NC_CACHE, LAST_RESULT
    x = np.asarray(x, dtype=np.float32)
    W1 = np.asarray(W1, dtype=np.float32)
    W2 = np.asarray(W2, dtype=np.float32)

    X = x.reshape(M_TOTAL, E)
    XT8 = np.ascontiguousarray(X.T).astype(_E4)  # [E, M_TOTAL]
    w1t, w2t, w2sum8, biasd = _prep_weights(W1, W2)
    sel8, selt8 = _selectors()

    in_maps = []
    for c in range(N_CORES):
        xt_c = np.ascontiguousarray(XT8[:, c * M_CORE:(c + 1) * M_CORE])
        in_maps.append(
            {"xt": xt_c, "w1t": w1t, "w2t": w2t, "sel8": sel8,
             "selt8": selt8, "w2sum8": w2sum8, "biasd": biasd}
        )

    if _NC_CACHE is None:
        _NC_CACHE = build_nc()

    if trace:
        trace = _ensure_ntff_hook()

    res = run_bass_kernel_spmd(_NC_CACHE, in_maps, list(range(N_CORES)), trace=trace)
    LAST_RESULT = res

    OT = np.concatenate(
        [np.asarray(res.results[c]["outT"]).astype(np.float32) for c in range(N_CORES)],
        axis=1,
    )
    return np.ascontiguousarray(OT.T).reshape(B, S, E)


# revision 11
# speedup vs baseline: 1.6518x; 1.0384x over previous
"""Trainium2 Bass kernel for nn_Attention_9242769622327.

Math: the reference computes
    qkv = x @ W1.T ; q,k,v = split(qkv)
    score = softmax(k^T v / 4, axis=-1)            # rows sum to 1
    attn  = softmax(einsum('bhnk,bhkc->bhnk', q/4, score), axis=-1)
          = softmax(q/4)                           # k/v are mathematically dead
    out   = attn @ W2.T
so only the q-projection (first E rows of W1), a per-head (64-wide) softmax,
and the output projection are needed.

Distribution: pure data-parallel over the 32768 = B*S rows; each of the 8
cores handles 4096 rows with the full (transposed) weights. No collectives.

EVERY matmul runs as fp8 e4m3 MatmulPerfMode.DoubleRow (two K-tiles per
instruction, 2x PE throughput; measured 216ns per [*,2,*]x[*,2,512]
instruction back-to-back — and mixing any f16 matmul into the fp8 stream
was measured to stall the PE at ~2x cycle time for ~us, so the PE stream
is kept mode-pure).  fp8's ~2.7% per-value quantization noise is tamed by
centering every fp8-quantized quantity around its known mean:
  - mm1 (q-projection): softmax's /4 temperature + normalization damp the
    error ~4x; plain fp8 x / fp8 (32*W1q) gives ~0.9% final.
  - attn: rows sum to exactly 1 per 64-wide head, so the PE gets
    at0 = (u - 1)*rb   (u = exp(q/4) f16, rb = A*64/s broadcast, A=16)
    which is ~4x smaller than u*rb.
  - rcp: rcpc = A*64/s - A (+-2.5) is quantized fp8; the exact constant A
    is restored through a second K-tile whose selector row multiplies a
    constant-A row, so rb = selt0^T@rcpc + A exactly in fp32 PSUM.
  - mm2 constant part: sum_h rcp*w2sum splits into the exact f32 bias
    colsum(32*W2T)/2048 (applied in the output-copy ACT) plus the tiny
    centered fp8 matmul rcpc @ w2sum8.
  - head sums: s comes from an fp8 copy of u (DVE); the 2.7%/sqrt(64)
    coherent error this adds is ~0.3%.
Host-emulated + CoreSim rel err: 1.35e-2 (gate 2e-2).

On-chip layout is fully transposed (features on partitions, rows on the
free dim) so no on-chip transposes are needed anywhere:
    qT[n,m]  = sum_k W1qT[k,n]*xT[k,m]        (PE DR, fp8, PSUM=32q)
    u        = exp(qT/128)                    (ACT, PSUM->SBUF f16)
    u8       = fp8(u)                         (DVE copy, pair tiles)
    s[g,m]/64= sum_{n in g} u8[n,m]/64        (PE DR w/ 1/64 selector pairs)
    rcpc     = fp8(A*64/s - A)                (DVE reciprocal + tensor_scalar)
    rb[n,m]  = rcpc[head(n),m] + A            (PE DR w/ selector^T + const row)
    at0      = (u - 1)*rb                     (DVE scalar_tensor_tensor, fp8)
    outT[j,m]= [rcpc@w2sum8 + sum_n W2T[n,j]*at0[n,m]]/(A*64*32) + bias_j
                                              (PE DR; ACT Identity w/ bias)

Per stripe the PE issues 84 uniform DR matmuls: 8 rb (interleaved into the
mm1 loop so the PE stays ahead of the DVE's at0 chain), 32 mm1, 40 mm2
(8 j-groups of [rcpc-corr + 4 at0]), 4 sel — ~218ns each sustained.
"""

import sys

sys.path.insert(0, "/opt/trn_rl_repo")

import numpy as np
import ml_dtypes

import concourse.bass as bass
import concourse.bacc as bacc
import concourse.tile as tile
from concourse import mybir
from concourse.bass_utils import run_bass_kernel_spmd

F16 = mybir.dt.float16
FP8 = mybir.dt.float8e4
F32 = mybir.dt.float32
AF = mybir.ActivationFunctionType
ALU = mybir.AluOpType
DR = mybir.MatmulPerfMode.DoubleRow

N_CORES = 8
B, S, E = 4, 8192, 1024
HEADS, HEAD_DIM = 16, 64
M_TOTAL = B * S                # 32768
M_CORE = M_TOTAL // N_CORES    # 4096 rows per core
MS = 512                       # m-stripe width (moving free dim / PSUM bank)
KC2 = E // 256                 # 4 DoubleRow contraction pairs
NC_ = E // 128                 # 8 feature chunks
A_SCALE = 16.0                 # fp8 scale for the centered attn
W_SCALE = 32.0                 # host pre-scale of W1q/W2 (std 1/32 -> ~1)
OUT_DESCALE = 1.0 / (A_SCALE * 64.0 * W_SCALE)

_E4 = ml_dtypes.float8_e4m3
_F16 = np.float16


def build_nc(m_core=M_CORE) -> bass.Bass:
    n_stripes = m_core // MS
    nc = bacc.Bacc("TRN2", debug=False)

    xt = nc.dram_tensor("xt", [E, m_core], FP8, kind="ExternalInput")
    w1t = nc.dram_tensor("w1t", [E, E], FP8, kind="ExternalInput")
    w2t = nc.dram_tensor("w2t", [E, E], FP8, kind="ExternalInput")
    sel8 = nc.dram_tensor("sel8", [128, KC2 * 2 * 128], FP8, kind="ExternalInput")
    selt8 = nc.dram_tensor("selt8", [128, NC_ * 2 * 128], FP8, kind="ExternalInput")
    w2sum8 = nc.dram_tensor("w2sum8", [128, NC_ * 2 * 128], FP8, kind="ExternalInput")
    biasd = nc.dram_tensor("biasd", [128, NC_], F32, kind="ExternalInput")
    outT = nc.dram_tensor("outT", [E, m_core], F16, kind="ExternalOutput")

    xt_v = xt[:, :].rearrange("(c p) m -> p c m", p=128)    # [128, 8, m_core]
    w1_v = w1t[:, :].rearrange("(c p) n -> p c n", p=128)   # [128, 8, 1024]
    w2_v = w2t[:, :].rearrange("(c p) j -> p c j", p=128)   # [128, 8, 1024]

    with tile.TileContext(nc) as tc:
        with (
            tc.tile_pool(name="weights", bufs=1) as wpool,
            tc.tile_pool(name="xt", bufs=n_stripes) as xpool,
            tc.tile_pool(name="u", bufs=16) as upool,
            tc.tile_pool(name="u8", bufs=4) as u8pool,
            tc.tile_pool(name="at", bufs=8) as apool,
            tc.tile_pool(name="small", bufs=2) as spool,
            tc.tile_pool(name="ostage", bufs=8) as opool,
            tc.tile_pool(name="ps_q", bufs=3, space="PSUM") as psq,
            tc.tile_pool(name="ps_s", bufs=1, space="PSUM") as pss,
            tc.tile_pool(name="ps_rb", bufs=2, space="PSUM") as psrb,
            tc.tile_pool(name="ps_o", bufs=2, space="PSUM") as pso,
        ):
            # Warm the PE's HAM clock gate with throwaway DR matmuls (same
            # mode as the real stream) while the first DMAs are in flight.
            # memset on DVE: gpsimd's library load would delay it ~7us.
            warm_sb = wpool.tile([128, 2, MS], FP8, name="warm_sb")
            nc.vector.memset(warm_sb[:], 0.0)
            warm_ps = psq.tile([128, MS], F32, tag="q", name="warm_ps")
            for _ in range(16):
                nc.tensor.matmul(
                    warm_ps[:], warm_sb[:, :, 0:128], warm_sb[:],
                    start=True, stop=True, perf_mode=DR,
                )

            # Weight/selector loads, ordered so stripe 0's mm1 can start
            # as early as possible.
            w1_k = []   # 4 tiles [128, 2, E] fp8 (DoubleRow pairs)
            x_k = [[None] * KC2 for _ in range(n_stripes)]
            for kc2 in range(KC2):
                t = wpool.tile([128, 2, E], FP8, name=f"w1k{kc2}")
                nc.sync.dma_start(t[:], w1_v[:, 2 * kc2:2 * kc2 + 2, :])
                w1_k.append(t)
                tx = xpool.tile([128, 2, MS], FP8, tag=f"xt_{kc2}", name=f"xt0_{kc2}")
                nc.sync.dma_start(tx[:], xt_v[:, 2 * kc2:2 * kc2 + 2, 0:MS])
                x_k[0][kc2] = tx
            sel8_t = wpool.tile([128, KC2, 2, 128], FP8, name="sel8_t")
            nc.sync.dma_start(
                sel8_t[:], sel8[:, :].rearrange("p (c i g) -> p c i g", i=2, g=128)
            )
            selt8_t = wpool.tile([128, NC_, 2, 128], FP8, name="selt8_t")
            nc.sync.dma_start(
                selt8_t[:], selt8[:, :].rearrange("p (c i q) -> p c i q", i=2, q=128)
            )
            w2sum8_t = wpool.tile([128, NC_, 2, 128], FP8, name="w2sum8_t")
            nc.sync.dma_start(
                w2sum8_t[:], w2sum8[:, :].rearrange("p (c i q) -> p c i q", i=2, q=128)
            )
            bias_t = wpool.tile([128, NC_], F32, name="bias_t")
            nc.sync.dma_start(bias_t[:], biasd[:, :])

            w2_k = []   # 4 tiles [128, 2, E] fp8 (pairs of n-chunks)
            for c2 in range(KC2):
                t = wpool.tile([128, 2, E], FP8, name=f"w2k{c2}")
                nc.sync.dma_start(t[:], w2_v[:, 2 * c2:2 * c2 + 2, :])
                w2_k.append(t)

            def prefetch_x(ms):
                if ms < 1 or ms >= n_stripes:
                    return
                for kc2 in range(KC2):
                    tx = xpool.tile(
                        [128, 2, MS], FP8, tag=f"xt_{kc2}", name=f"xt{ms}_{kc2}"
                    )
                    nc.sync.dma_start(
                        tx[:], xt_v[:, 2 * kc2:2 * kc2 + 2, ms * MS:(ms + 1) * MS]
                    )
                    x_k[ms][kc2] = tx
            prefetch_x(1)

            # rcpc tiles [128, 2, MS] fp8 (K padded to 128 so every matmul
            # in the stream has the same (128,128) tile shape — K=16 tiles
            # measured +285ns/group): [0:16,0,:] = centered reciprocal
            # (DVE-written per stripe), [0,1,:] = A (the decenter constant,
            # written once), rest zero.  Two alternating buffers.
            rcpc_bufs = []
            for i in range(2):
                t = wpool.tile([128, 2, MS], FP8, name=f"rcpc{i}")
                nc.gpsimd.memset(t[:], 0.0)
                nc.gpsimd.memset(t[0:1, 1, :], A_SCALE)
                rcpc_bufs.append(t)

            def emit_rb_at0(prev, ci):
                """rb broadcast DR matmul + fused (u-1)*rb DVE op -> fp8."""
                rb_ps = psrb.tile([128, MS], F32, tag="rb", name="rb_ps")
                nc.tensor.matmul(
                    rb_ps[:], selt8_t[:, ci, :, :], prev["rcpc"][:],
                    start=True, stop=True, perf_mode=DR,
                )
                pair, half = divmod(ci, 2)
                nc.vector.scalar_tensor_tensor(
                    prev["at"][pair][:, half, :],
                    prev["u"][ci][:],
                    -1.0,
                    rb_ps[:],
                    op0=ALU.add,
                    op1=ALU.mult,
                )

            def emit_mm2(prev, js):
                """j-groups: [rcpc-corr DR (K=16), 4 fp8 DR at0 matmuls]."""
                for j in js:
                    o_ps = pso.tile([128, MS], F32, tag="o", name="o_ps")
                    nc.tensor.matmul(
                        o_ps[:],
                        w2sum8_t[:, j, :, :],
                        prev["rcpc"][:],
                        start=True,
                        stop=False,
                        perf_mode=DR,
                        skip_group_check=True,
                    )
                    for c2 in range(KC2):
                        nc.tensor.matmul(
                            o_ps[:],
                            w2_k[c2][:, :, j * 128:(j + 1) * 128],
                            prev["at"][c2][:],
                            start=False,
                            stop=(c2 == KC2 - 1),
                            perf_mode=DR,
                            skip_group_check=True,
                        )
                    o_t = opool.tile([128, MS], F16, tag="ost", name="o_t")
                    nc.scalar.activation(
                        o_t[:], o_ps[:], AF.Identity,
                        bias=bias_t[:, j:j + 1], scale=OUT_DESCALE,
                    )
                    nc.sync.dma_start(
                        outT[j * 128:(j + 1) * 128,
                             prev["ms"] * MS:(prev["ms"] + 1) * MS],
                        o_t[:],
                    )

            def emit_sel(cur):
                """4 DR head-sum matmuls on the fp8 u pairs + rcp chain."""
                s_ps = pss.tile([128, MS], F32, tag="s", name="s_ps")
                for c2 in range(KC2):
                    nc.tensor.matmul(
                        s_ps[:],
                        sel8_t[:, c2, :, :],
                        cur["u8"][c2][:],
                        start=(c2 == 0),
                        stop=(c2 == KC2 - 1),
                        perf_mode=DR,
                    )
                rcp32 = spool.tile([HEADS, MS], F32, tag="rcp32", name="rcp32")
                nc.vector.reciprocal_approx_fast(rcp32[:], s_ps[0:HEADS, :])
                nc.vector.tensor_scalar(
                    cur["rcpc"][0:HEADS, 0, :], rcp32[:], A_SCALE, A_SCALE,
                    op0=ALU.mult, op1=ALU.subtract,
                )

            prev = None
            for ms in range(n_stripes):
                prefetch_x(ms + 1)
                cur = {
                    "ms": ms,
                    "u": [],
                    "u8": [
                        u8pool.tile([128, 2, MS], FP8, tag=f"u8{p}", name=f"u8{ms}_{p}")
                        for p in range(KC2)
                    ],
                    "at": [
                        apool.tile([128, 2, MS], FP8, tag=f"at{p}", name=f"at{ms}_{p}")
                        for p in range(KC2)
                    ],
                    "rcpc": rcpc_bufs[ms % 2],
                }

                # Head of block: 2 rb matmuls of the previous stripe; the
                # rest interleave into the mm1 loop so the PE stays just
                # ahead of the DVE's 823ns/op at0 chain (psrb has 2 bufs).
                if prev is not None:
                    emit_rb_at0(prev, 0)
                    emit_rb_at0(prev, 1)

                for ci in range(NC_):
                    q_ps = psq.tile([128, MS], F32, tag="q", name="q_ps")
                    for kc2 in range(KC2):
                        nc.tensor.matmul(
                            q_ps[:],
                            w1_k[kc2][:, :, ci * 128:(ci + 1) * 128],
                            x_k[ms][kc2][:],
                            start=(kc2 == 0),
                            stop=(kc2 == KC2 - 1),
                            perf_mode=DR,
                        )
                    u_t = upool.tile([128, MS], F16, tag="u", name="u_t")
                    nc.scalar.activation(u_t[:], q_ps[:], AF.Exp, scale=1.0 / 128.0)
                    cur["u"].append(u_t)
                    if prev is not None and ci + 2 < NC_:
                        emit_rb_at0(prev, ci + 2)

                # u8 copies (DVE) emitted after all at0 ops so the at0
                # chain (needed first, by mm2) drains first.
                for ci in range(NC_):
                    nc.vector.tensor_copy(
                        cur["u8"][ci // 2][:, ci % 2, :], cur["u"][ci][:]
                    )

                if prev is not None:
                    emit_mm2(prev, range(6))
                emit_sel(cur)
                if prev is not None:
                    emit_mm2(prev, range(6, NC_))
                prev = cur

            # epilogue: last stripe's rb/at0 + output projection
            for ci in range(NC_):
                emit_rb_at0(prev, ci)
            emit_mm2(prev, range(NC_))
    nc.compile()
    return nc


_NC_CACHE = None
LAST_RESULT = None


def _ensure_ntff_hook():
    """bass_utils' axon trace path needs antenv.axon_hooks, which this
    container's antenv lacks. Provide it + register the ctypes NTFF hook."""
    import types

    try:
        from antenv.axon_hooks import get_axon_ntff_profile_hook  # noqa: F401
        return True
    except ImportError:
        pass
    try:
        import antenv
        from trn_agent_boot.trn_boot import _ntff_profile_via_ctypes

        m = types.ModuleType("antenv.axon_hooks")
        state = {"hook": None}
        m.set_axon_ntff_profile_hook = lambda h: state.__setitem__("hook", h)
        m.get_axon_ntff_profile_hook = lambda: state["hook"]
        sys.modules["antenv.axon_hooks"] = m
        antenv.axon_hooks = m
        m.set_axon_ntff_profile_hook(
            _ntff_profile_via_ctypes("/opt/axon/libaxon_pjrt.so")
        )
        return True
    except Exception as e:  # pragma: no cover
        print(f"ntff hook injection failed: {e}")
        return False


def _selectors():
    # head index of global feature n is n // 64; chunk ci covers n in
    # [128ci, 128ci+128) -> heads 2ci (partitions 0..63), 2ci+1 (64..127)
    # All selector tiles are padded to the uniform (128, 2, 128) shape.
    sel8 = np.zeros((128, KC2, 2, 128), np.float32)
    for c2 in range(KC2):
        for i in range(2):
            ci = 2 * c2 + i
            sel8[:64, c2, i, 2 * ci] = 1.0 / 64.0
            sel8[64:, c2, i, 2 * ci + 1] = 1.0 / 64.0
    # selt8[:, ci, 0, :]: 0/1 head selector; [:, ci, 1, :]: row 0 = 1.0,
    # which multiplies the constant-A row of rcpc_pad -> rb = rcpc + A.
    selt8 = np.zeros((128, NC_, 2, 128), np.float32)
    for ci in range(NC_):
        selt8[2 * ci, ci, 0, :64] = 1.0
        selt8[2 * ci + 1, ci, 0, 64:] = 1.0
        selt8[0, ci, 1, :] = 1.0
    return (
        np.ascontiguousarray(sel8.reshape(128, KC2 * 2 * 128)).astype(_E4),
        np.ascontiguousarray(selt8.reshape(128, NC_ * 2 * 128)).astype(_E4),
    )


def _prep_weights(W1, W2):
    w1t = np.ascontiguousarray(W1[:E, :].T * W_SCALE).astype(_E4)  # [k, n]
    w2t = np.ascontiguousarray(W2.T * W_SCALE).astype(_E4)         # [n, j]
    # exact per-head column sums of 32*W2T (fp32)
    w2sum32 = (W2.T * W_SCALE).reshape(HEADS, HEAD_DIM, E).sum(axis=1)
    # w2sum8[0:16, j, 0, :] = fp8 per-head colsums (multiplies rcpc);
    # [:, j, 1, :] = 0 (the exact constant part goes through bias).
    w2sum8 = np.zeros((128, NC_, 2, 128), np.float32)
    for j in range(NC_):
        w2sum8[:HEADS, j, 0, :] = w2sum32[:, j * 128:(j + 1) * 128]
    bias = (w2sum32.sum(axis=0) * A_SCALE * OUT_DESCALE).astype(np.float32)
    biasd = np.ascontiguousarray(bias.reshape(NC_, 128).T)         # [128, NC_]
    return (
        w1t,
        w2t,
        np.ascontiguousarray(w2sum8.reshape(128, NC_ * 2 * 128)).astype(_E4),
        biasd,
    )


def kernel(x, W1, W2, heads, trace=False):
    global _NC_CACHE, LAST_RESULT
    x = np.asarray(x, dtype=np.float32)
    W1 = np.asarray(W1, dtype=np.float32)
    W2 = np.asarray(W2, dtype=np.float32)

    X = x.reshape(M_TOTAL, E)
    XT8 = np.ascontiguousarray(X.T).astype(_E4)  # [E, M_TOTAL]
    w1t, w2t, w2sum8, biasd = _prep_weights(W1, W2)
    sel8, selt8 = _selectors()

    in_maps = []
    for c in range(N_CORES):
        xt_c = np.ascontiguousarray(XT8[:, c * M_CORE:(c + 1) * M_CORE])
        in_maps.append(
            {"xt": xt_c, "w1t": w1t, "w2t": w2t, "sel8": sel8,
             "selt8": selt8, "w2sum8": w2sum8, "biasd": biasd}
        )

    if _NC_CACHE is None:
        _NC_CACHE = build_nc()

    if trace:
        trace = _ensure_ntff_hook()

    res = run_bass_kernel_spmd(_NC_CACHE, in_maps, list(range(N_CORES)), trace=trace)
    LAST_RESULT = res

    OT = np.concatenate(
        [np.asarray(res.results[c]["outT"]).astype(np.float32) for c in range(N_CORES)],
        axis=1,
    )
    return np.ascontiguousarray(OT.T).reshape(B, S, E)


# revision 13
# speedup vs baseline: 1.9363x; 1.1722x over previous
"""Trainium2 Bass kernel for nn_Attention_9242769622327.

Math: the reference computes
    qkv = x @ W1.T ; q,k,v = split(qkv)
    score = softmax(k^T v / 4, axis=-1)            # rows sum to 1
    attn  = softmax(einsum('bhnk,bhkc->bhnk', q/4, score), axis=-1)
          = softmax(q/4)                           # k/v are mathematically dead
    out   = attn @ W2.T
so only the q-projection (first E rows of W1), a per-head (64-wide) softmax,
and the output projection are needed.

Distribution: pure data-parallel over the 32768 = B*S rows; each of the 8
cores handles 4096 rows with the full (transposed) weights. No collectives.

EVERY matmul runs as fp8 e4m3 MatmulPerfMode.DoubleRow (two K-tiles per
instruction, 2x PE throughput; measured 216ns per [*,2,*]x[*,2,512]
instruction back-to-back — and mixing any f16 matmul into the fp8 stream
was measured to stall the PE at ~2x cycle time for ~us, so the PE stream
is kept mode-pure).  fp8's ~2.7% per-value quantization noise is tamed by
centering every fp8-quantized quantity around its known mean:
  - mm1 (q-projection): softmax's /4 temperature + normalization damp the
    error ~4x; plain fp8 x / fp8 (32*W1q) gives ~0.9% final.
  - attn: rows sum to exactly 1 per 64-wide head, so the PE gets
    at0 = (u - 1)*rb   (u = exp(q/4) f16, rb = A*64/s broadcast, A=16)
    which is ~4x smaller than u*rb.
  - rcp: rcpc = A*64/s - A (+-2.5) is quantized fp8; the exact constant A
    is restored through a second K-tile whose selector row multiplies a
    constant-A row, so rb = selt0^T@rcpc + A exactly in fp32 PSUM.
  - mm2 constant part: sum_h rcp*w2sum splits into the exact f32 bias
    colsum(32*W2T)/2048 (applied in the output-copy ACT) plus the tiny
    centered fp8 matmul rcpc @ w2sum8.
  - head sums: s comes from an fp8 copy of u (DVE); the 2.7%/sqrt(64)
    coherent error this adds is ~0.3%.
Host-emulated + CoreSim rel err: 1.35e-2 (gate 2e-2).

On-chip layout is fully transposed (features on partitions, rows on the
free dim) so no on-chip transposes are needed anywhere:
    qT[n,m]  = sum_k W1qT[k,n]*xT[k,m]        (PE DR, fp8, PSUM=32q)
    u        = exp(qT/128)                    (ACT, PSUM->SBUF f16)
    u8       = fp8(u)                         (DVE copy, pair tiles)
    s[g,m]/64= sum_{n in g} u8[n,m]/64        (PE DR w/ 1/64 selector pairs)
    rcpc     = fp8(A*64/s - A)                (DVE reciprocal + tensor_scalar)
    rb[n,m]  = rcpc[head(n),m] + A            (PE DR w/ selector^T + const row)
    at0      = (u - 1)*rb                     (DVE scalar_tensor_tensor, fp8)
    outT[j,m]= [rcpc@w2sum8 + sum_n W2T[n,j]*at0[n,m]]/(A*64*32) + bias_j
                                              (PE DR; ACT Identity w/ bias)

Per stripe the PE issues 84 uniform DR matmuls: 8 rb (interleaved into the
mm1 loop so the PE stays ahead of the DVE's at0 chain), 32 mm1, 40 mm2
(8 j-groups of [rcpc-corr + 4 at0]), 4 sel — ~218ns each sustained.
"""

import sys

sys.path.insert(0, "/opt/trn_rl_repo")

import numpy as np
import ml_dtypes

import concourse.bass as bass
import concourse.bacc as bacc
import concourse.tile as tile
from concourse import mybir
from concourse.bass_utils import run_bass_kernel_spmd

F16 = mybir.dt.float16
FP8 = mybir.dt.float8e4
F32 = mybir.dt.float32
AF = mybir.ActivationFunctionType
ALU = mybir.AluOpType
DR = mybir.MatmulPerfMode.DoubleRow

N_CORES = 8
B, S, E = 4, 8192, 1024
HEADS, HEAD_DIM = 16, 64
M_TOTAL = B * S                # 32768
M_CORE = M_TOTAL // N_CORES    # 4096 rows per core
MS = 512                       # m-stripe width (moving free dim / PSUM bank)
KC2 = E // 256                 # 4 DoubleRow contraction pairs
NC_ = E // 128                 # 8 feature chunks
A_SCALE = 16.0                 # fp8 scale for the centered attn
W_SCALE = 32.0                 # host pre-scale of W1q/W2 (std 1/32 -> ~1)
OUT_DESCALE = 1.0 / (A_SCALE * 64.0 * W_SCALE)

_E4 = ml_dtypes.float8_e4m3
_F16 = np.float16


def build_nc(m_core=M_CORE) -> bass.Bass:
    n_stripes = m_core // MS
    nc = bacc.Bacc("TRN2", debug=False)

    xt = nc.dram_tensor("xt", [E, m_core], FP8, kind="ExternalInput")
    w1t = nc.dram_tensor("w1t", [E, E], FP8, kind="ExternalInput")
    w2t = nc.dram_tensor("w2t", [E, E], FP8, kind="ExternalInput")
    sel8 = nc.dram_tensor("sel8", [128, KC2 * 2 * 128], FP8, kind="ExternalInput")
    selt8 = nc.dram_tensor("selt8", [128, NC_ * 2 * 128], FP8, kind="ExternalInput")
    w2sum8 = nc.dram_tensor("w2sum8", [128, NC_ * 2 * 128], FP8, kind="ExternalInput")
    biasd = nc.dram_tensor("biasd", [128, NC_], F32, kind="ExternalInput")
    outT = nc.dram_tensor("outT", [E, m_core], F16, kind="ExternalOutput")

    xt_v = xt[:, :].rearrange("(c p) m -> p c m", p=128)    # [128, 8, m_core]
    w1_v = w1t[:, :].rearrange("(c p) n -> p c n", p=128)   # [128, 8, 1024]
    w2_v = w2t[:, :].rearrange("(c p) j -> p c j", p=128)   # [128, 8, 1024]

    with tile.TileContext(nc) as tc:
        with (
            tc.tile_pool(name="weights", bufs=1) as wpool,
            tc.tile_pool(name="xt", bufs=n_stripes) as xpool,
            tc.tile_pool(name="u", bufs=16) as upool,
            tc.tile_pool(name="u8", bufs=4) as u8pool,
            tc.tile_pool(name="at", bufs=8) as apool,
            tc.tile_pool(name="small", bufs=2) as spool,
            tc.tile_pool(name="ostage", bufs=8) as opool,
            tc.tile_pool(name="ps_q", bufs=3, space="PSUM") as psq,
            tc.tile_pool(name="ps_s", bufs=1, space="PSUM") as pss,
            tc.tile_pool(name="ps_rb", bufs=2, space="PSUM") as psrb,
            tc.tile_pool(name="ps_o", bufs=2, space="PSUM") as pso,
        ):
            # Warm the PE's HAM clock gate with small throwaway DR matmuls
            # (same mode as the real stream) while the first DMAs are in
            # flight.  gpsimd memset is ready right after its preamble
            # (~6.5us), before the PE's own preamble ends; N=128 keeps each
            # cold-clock warm matmul cheap so they don't delay stripe 0.
            warm_sb = wpool.tile([128, 2, 128], FP8, name="warm_sb")
            nc.gpsimd.memset(warm_sb[:], 0.0)
            warm_ps = psq.tile([128, MS], F32, tag="q", name="warm_ps")
            for _ in range(10):
                nc.tensor.matmul(
                    warm_ps[:, 0:128], warm_sb[:], warm_sb[:],
                    start=True, stop=True, perf_mode=DR,
                )

            # Weight/selector loads, ordered so stripe 0's mm1 can start
            # as early as possible.
            w1_k = []   # 4 tiles [128, 2, E] fp8 (DoubleRow pairs)
            x_k = [[None] * KC2 for _ in range(n_stripes)]
            for kc2 in range(KC2):
                t = wpool.tile([128, 2, E], FP8, name=f"w1k{kc2}")
                nc.sync.dma_start(t[:], w1_v[:, 2 * kc2:2 * kc2 + 2, :])
                w1_k.append(t)
                tx = xpool.tile([128, 2, MS], FP8, tag=f"xt_{kc2}", name=f"xt0_{kc2}")
                nc.sync.dma_start(tx[:], xt_v[:, 2 * kc2:2 * kc2 + 2, 0:MS])
                x_k[0][kc2] = tx
            sel8_t = wpool.tile([128, KC2, 2, 128], FP8, name="sel8_t")
            nc.sync.dma_start(
                sel8_t[:], sel8[:, :].rearrange("p (c i g) -> p c i g", i=2, g=128)
            )
            selt8_t = wpool.tile([128, NC_, 2, 128], FP8, name="selt8_t")
            nc.sync.dma_start(
                selt8_t[:], selt8[:, :].rearrange("p (c i q) -> p c i q", i=2, q=128)
            )
            w2sum8_t = wpool.tile([128, NC_, 2, 128], FP8, name="w2sum8_t")
            nc.sync.dma_start(
                w2sum8_t[:], w2sum8[:, :].rearrange("p (c i q) -> p c i q", i=2, q=128)
            )
            bias_t = wpool.tile([128, NC_], F32, name="bias_t")
            nc.sync.dma_start(bias_t[:], biasd[:, :])

            w2_k = []   # 4 tiles [128, 2, E] fp8 (pairs of n-chunks)
            for c2 in range(KC2):
                t = wpool.tile([128, 2, E], FP8, name=f"w2k{c2}")
                nc.sync.dma_start(t[:], w2_v[:, 2 * c2:2 * c2 + 2, :])
                w2_k.append(t)

            def prefetch_x(ms):
                if ms < 1 or ms >= n_stripes:
                    return
                for kc2 in range(KC2):
                    tx = xpool.tile(
                        [128, 2, MS], FP8, tag=f"xt_{kc2}", name=f"xt{ms}_{kc2}"
                    )
                    nc.sync.dma_start(
                        tx[:], xt_v[:, 2 * kc2:2 * kc2 + 2, ms * MS:(ms + 1) * MS]
                    )
                    x_k[ms][kc2] = tx
            prefetch_x(1)

            # rcpc tiles [128, 2, MS] fp8 (K padded to 128 so every matmul
            # in the stream has the same (128,128) tile shape — K=16 tiles
            # measured +285ns/group): [0:16,0,:] = centered reciprocal
            # (DVE-written per stripe), [0,1,:] = A (the decenter constant,
            # written once), rest zero.  Two alternating buffers.
            rcpc_bufs = []
            for i in range(2):
                t = wpool.tile([128, 2, MS], FP8, name=f"rcpc{i}")
                nc.gpsimd.memset(t[:], 0.0)
                nc.gpsimd.memset(t[0:1, 1, :], A_SCALE)
                rcpc_bufs.append(t)

            def emit_rb_at0(prev, ci):
                """rb broadcast DR matmul + fused (u-1)*rb DVE op -> fp8."""
                rb_ps = psrb.tile([128, MS], F32, tag="rb", name="rb_ps")
                nc.tensor.matmul(
                    rb_ps[:], selt8_t[:, ci, :, :], prev["rcpc"][:],
                    start=True, stop=True, perf_mode=DR,
                )
                pair, half = divmod(ci, 2)
                nc.vector.scalar_tensor_tensor(
                    prev["at"][pair][:, half, :],
                    prev["u"][ci][:],
                    -1.0,
                    rb_ps[:],
                    op0=ALU.add,
                    op1=ALU.mult,
                )

            def emit_mm2(prev, js):
                """j-groups: [rcpc-corr DR (K=16), 4 fp8 DR at0 matmuls]."""
                for j in js:
                    o_ps = pso.tile([128, MS], F32, tag="o", name="o_ps")
                    nc.tensor.matmul(
                        o_ps[:],
                        w2sum8_t[:, j, :, :],
                        prev["rcpc"][:],
                        start=True,
                        stop=False,
                        perf_mode=DR,
                        skip_group_check=True,
                    )
                    for c2 in range(KC2):
                        nc.tensor.matmul(
                            o_ps[:],
                            w2_k[c2][:, :, j * 128:(j + 1) * 128],
                            prev["at"][c2][:],
                            start=False,
                            stop=(c2 == KC2 - 1),
                            perf_mode=DR,
                            skip_group_check=True,
                        )
                    o_t = opool.tile([128, MS], F16, tag="ost", name="o_t")
                    nc.scalar.activation(
                        o_t[:], o_ps[:], AF.Identity,
                        bias=bias_t[:, j:j + 1], scale=OUT_DESCALE,
                    )
                    nc.sync.dma_start(
                        outT[j * 128:(j + 1) * 128,
                             prev["ms"] * MS:(prev["ms"] + 1) * MS],
                        o_t[:],
                    )

            def emit_sel(cur):
                """4 DR head-sum matmuls on the fp8 u pairs + rcp chain."""
                s_ps = pss.tile([128, MS], F32, tag="s", name="s_ps")
                for c2 in range(KC2):
                    nc.tensor.matmul(
                        s_ps[:],
                        sel8_t[:, c2, :, :],
                        cur["u8"][c2][:],
                        start=(c2 == 0),
                        stop=(c2 == KC2 - 1),
                        perf_mode=DR,
                    )
                rcp32 = spool.tile([HEADS, MS], F32, tag="rcp32", name="rcp32")
                nc.vector.reciprocal_approx_fast(rcp32[:], s_ps[0:HEADS, :])
                nc.vector.tensor_scalar(
                    cur["rcpc"][0:HEADS, 0, :], rcp32[:], A_SCALE, A_SCALE,
                    op0=ALU.mult, op1=ALU.subtract,
                )

            prev = None
            for ms in range(n_stripes):
                prefetch_x(ms + 1)
                cur = {
                    "ms": ms,
                    "u": [],
                    "u8": [
                        u8pool.tile([128, 2, MS], FP8, tag=f"u8{p}", name=f"u8{ms}_{p}")
                        for p in range(KC2)
                    ],
                    "at": [
                        apool.tile([128, 2, MS], FP8, tag=f"at{p}", name=f"at{ms}_{p}")
                        for p in range(KC2)
                    ],
                    "rcpc": rcpc_bufs[ms % 2],
                }

                # Head of block: 2 rb matmuls of the previous stripe; the
                # rest interleave into the mm1 loop so the PE stays just
                # ahead of the DVE's 823ns/op at0 chain (psrb has 2 bufs).
                if prev is not None:
                    emit_rb_at0(prev, 0)
                    emit_rb_at0(prev, 1)

                for ci in range(NC_):
                    q_ps = psq.tile([128, MS], F32, tag="q", name="q_ps")
                    for kc2 in range(KC2):
                        nc.tensor.matmul(
                            q_ps[:],
                            w1_k[kc2][:, :, ci * 128:(ci + 1) * 128],
                            x_k[ms][kc2][:],
                            start=(kc2 == 0),
                            stop=(kc2 == KC2 - 1),
                            perf_mode=DR,
                        )
                    u_t = upool.tile([128, MS], F16, tag="u", name="u_t")
                    nc.scalar.activation(u_t[:], q_ps[:], AF.Exp, scale=1.0 / 128.0)
                    cur["u"].append(u_t)
                    if prev is not None and ci + 2 < NC_:
                        emit_rb_at0(prev, ci + 2)

                # u8 copies (DVE) emitted after all at0 ops so the at0
                # chain (needed first, by mm2) drains first.
                for ci in range(NC_):
                    nc.vector.tensor_copy(
                        cur["u8"][ci // 2][:, ci % 2, :], cur["u"][ci][:]
                    )

                last = ms == n_stripes - 1
                if prev is not None:
                    emit_mm2(prev, range(4 if last else 6))
                emit_sel(cur)
                if prev is None:
                    pass
                elif not last:
                    emit_mm2(prev, range(6, NC_))
                else:
                    # Final block: interleave the last stripe's rb/at0 chain
                    # into the remaining mm2 groups of the previous stripe so
                    # the epilogue's DVE work overlaps PE work.
                    for k, j in enumerate(range(4, NC_)):
                        emit_rb_at0(cur, k)
                        emit_mm2(prev, [j])
                    for ci in range(4, NC_):
                        emit_rb_at0(cur, ci)
                prev = cur

            # epilogue: last stripe's output projection (or, single-stripe
            # runs, its rb/at0 chain too)
            if n_stripes == 1:
                for ci in range(NC_):
                    emit_rb_at0(prev, ci)
            emit_mm2(prev, range(NC_))
    nc.compile()
    return nc


_NC_CACHE = None
LAST_RESULT = None


def _ensure_ntff_hook():
    """bass_utils' axon trace path needs antenv.axon_hooks, which this
    container's antenv lacks. Provide it + register the ctypes NTFF hook."""
    import types

    try:
        from antenv.axon_hooks import get_axon_ntff_profile_hook  # noqa: F401
        return True
    except ImportError:
        pass
    try:
        import antenv
        from trn_agent_boot.trn_boot import _ntff_profile_via_ctypes

        m = types.ModuleType("antenv.axon_hooks")
        state = {"hook": None}
        m.set_axon_ntff_profile_hook = lambda h: state.__setitem__("hook", h)
        m.get_axon_ntff_profile_hook = lambda: state["hook"]
        sys.modules["antenv.axon_hooks"] = m
        antenv.axon_hooks = m
        m.set_axon_ntff_profile_hook(
            _ntff_profile_via_ctypes("/opt/axon/libaxon_pjrt.so")
        )
        return True
    except Exception as e:  # pragma: no cover
        print(f"ntff hook injection failed: {e}")
        return False


def _selectors():
    # head index of global feature n is n // 64; chunk ci covers n in
    # [128ci, 128ci+128) -> heads 2ci (partitions 0..63), 2ci+1 (64..127)
    # All selector tiles are padded to the uniform (128, 2, 128) shape.
    sel8 = np.zeros((128, KC2, 2, 128), np.float32)
    for c2 in range(KC2):
        for i in range(2):
            ci = 2 * c2 + i
            sel8[:64, c2, i, 2 * ci] = 1.0 / 64.0
            sel8[64:, c2, i, 2 * ci + 1] = 1.0 / 64.0
    # selt8[:, ci, 0, :]: 0/1 head selector; [:, ci, 1, :]: row 0 = 1.0,
    # which multiplies the constant-A row of rcpc_pad -> rb = rcpc + A.
    selt8 = np.zeros((128, NC_, 2, 128), np.float32)
    for ci in range(NC_):
        selt8[2 * ci, ci, 0, :64] = 1.0
        selt8[2 * ci + 1, ci, 0, 64:] = 1.0
        selt8[0, ci, 1, :] = 1.0
    return (
        np.ascontiguousarray(sel8.reshape(128, KC2 * 2 * 128)).astype(_E4),
        np.ascontiguousarray(selt8.reshape(128, NC_ * 2 * 128)).astype(_E4),
    )


def _prep_weights(W1, W2):
    w1t = np.ascontiguousarray(W1[:E, :].T * W_SCALE).astype(_E4)  # [k, n]
    w2t = np.ascontiguousarray(W2.T * W_SCALE).astype(_E4)         # [n, j]
    # exact per-head column sums of 32*W2T (fp32)
    w2sum32 = (W2.T * W_SCALE).reshape(HEADS, HEAD_DIM, E).sum(axis=1)
    # w2sum8[0:16, j, 0, :] = fp8 per-head colsums (multiplies rcpc);
    # [:, j, 1, :] = 0 (the exact constant part goes through bias).
    w2sum8 = np.zeros((128, NC_, 2, 128), np.float32)
    for j in range(NC_):
        w2sum8[:HEADS, j, 0, :] = w2sum32[:, j * 128:(j + 1) * 128]
    bias = (w2sum32.sum(axis=0) * A_SCALE * OUT_DESCALE).astype(np.float32)
    biasd = np.ascontiguousarray(bias.reshape(NC_, 128).T)         # [128, NC_]
    return (
        w1t,
        w2t,
        np.ascontiguousarray(w2sum8.reshape(128, NC_ * 2 * 128)).astype(_E4),
        biasd,
    )


def kernel(x, W1, W2, heads, trace=False):
    global _NC_CACHE, LAST_RESULT
    x = np.asarray(x, dtype=np.float32)
    W1 = np.asarray(W1, dtype=np.float32)
    W2 = np.asarray(W2, dtype=np.float32)

    X = x.reshape(M_TOTAL, E)
    XT8 = np.ascontiguousarray(X.T).astype(_E4)  # [E, M_TOTAL]
    w1t, w2t, w2sum8, biasd = _prep_weights(W1, W2)
    sel8, selt8 = _selectors()

    in_maps = []
    for c in range(N_CORES):
        xt_c = np.ascontiguousarray(XT8[:, c * M_CORE:(c + 1) * M_CORE])
        in_maps.append(
            {"xt": xt_c, "w1t": w1t, "w2t": w2t, "sel8": sel8,
             "selt8": selt8, "w2sum8": w2sum8, "biasd": biasd}
        )

    if _NC_CACHE is None:
        _NC_CACHE = build_nc()

    if trace:
        trace = _ensure_ntff_hook()

    res = run_bass_kernel_spmd(_NC_CACHE, in_maps, list(range(N_CORES)), trace=trace)
    LAST_RESULT = res

    OT = np.concatenate(
        [np.asarray(res.results[c]["outT"]).astype(np.float32) for c in range(N_CORES)],
        axis=1,
    )
    return np.ascontiguousarray(OT.T).reshape(B, S, E)


# revision 15
# speedup vs baseline: 1.9781x; 1.0216x over previous
"""Trainium2 Bass kernel for nn_Attention_9242769622327.

Math: the reference computes
    qkv = x @ W1.T ; q,k,v = split(qkv)
    score = softmax(k^T v / 4, axis=-1)            # rows sum to 1
    attn  = softmax(einsum('bhnk,bhkc->bhnk', q/4, score), axis=-1)
          = softmax(q/4)                           # k/v are mathematically dead
    out   = attn @ W2.T
so only the q-projection (first E rows of W1), a per-head (64-wide) softmax,
and the output projection are needed.

Distribution: pure data-parallel over the 32768 = B*S rows; each of the 8
cores handles 4096 rows with the full (transposed) weights. No collectives.

EVERY matmul runs as fp8 e4m3 MatmulPerfMode.DoubleRow (two K-tiles per
instruction, 2x PE throughput; measured 216ns per [*,2,*]x[*,2,512]
instruction back-to-back — and mixing any f16 matmul into the fp8 stream
was measured to stall the PE at ~2x cycle time for ~us, so the PE stream
is kept mode-pure).  fp8's ~2.7% per-value quantization noise is tamed by
centering every fp8-quantized quantity around its known mean:
  - mm1 (q-projection): softmax's /4 temperature + normalization damp the
    error ~4x; plain fp8 x / fp8 (32*W1q) gives ~0.9% final.
  - attn: rows sum to exactly 1 per 64-wide head, so the PE gets
    at0 = (u - 1)*rb   (u = exp(q/4) f16, rb = A*64/s broadcast, A=16)
    which is ~4x smaller than u*rb.
  - rcp: rcpc = A*64/s - A (+-2.5) is quantized fp8; the exact constant A
    is restored through a second K-tile whose selector row multiplies a
    constant-A row, so rb = selt0^T@rcpc + A exactly in fp32 PSUM.
  - mm2 constant part: sum_h rcp*w2sum splits into the exact f32 bias
    colsum(32*W2T)/2048 (applied in the output-copy ACT) plus the tiny
    centered fp8 matmul rcpc @ w2sum8.
  - head sums: s comes from an fp8 copy of u (DVE); the 2.7%/sqrt(64)
    coherent error this adds is ~0.3%.
Host-emulated + CoreSim rel err: 1.35e-2 (gate 2e-2).

On-chip layout is fully transposed (features on partitions, rows on the
free dim) so no on-chip transposes are needed anywhere:
    qT[n,m]  = sum_k W1qT[k,n]*xT[k,m]        (PE DR, fp8, PSUM=32q)
    u        = exp(qT/128)                    (ACT, PSUM->SBUF f16)
    u8       = fp8(u)                         (DVE copy, pair tiles)
    s[g,m]/64= sum_{n in g} u8[n,m]/64        (PE DR w/ 1/64 selector pairs)
    rcpc     = fp8(A*64/s - A)                (DVE reciprocal + tensor_scalar)
    rb[n,m]  = rcpc[head(n),m] + A            (PE DR w/ selector^T + const row)
    at0      = (u - 1)*rb                     (DVE scalar_tensor_tensor, fp8)
    outT[j,m]= [rcpc@w2sum8 + sum_n W2T[n,j]*at0[n,m]]/(A*64*32) + bias_j
                                              (PE DR; ACT Identity w/ bias)

Per stripe the PE issues 84 uniform DR matmuls: 8 rb (interleaved into the
mm1 loop so the PE stays ahead of the DVE's at0 chain), 32 mm1, 40 mm2
(8 j-groups of [rcpc-corr + 4 at0]), 4 sel — ~218ns each sustained.
"""

import sys

sys.path.insert(0, "/opt/trn_rl_repo")

import numpy as np
import ml_dtypes

import concourse.bass as bass
import concourse.bacc as bacc
import concourse.tile as tile
from concourse import mybir
from concourse.bass_utils import run_bass_kernel_spmd

F16 = mybir.dt.float16
FP8 = mybir.dt.float8e4
F32 = mybir.dt.float32
AF = mybir.ActivationFunctionType
ALU = mybir.AluOpType
DR = mybir.MatmulPerfMode.DoubleRow

N_CORES = 8
B, S, E = 4, 8192, 1024
HEADS, HEAD_DIM = 16, 64
M_TOTAL = B * S                # 32768
M_CORE = M_TOTAL // N_CORES    # 4096 rows per core
MS = 512                       # m-stripe width (moving free dim / PSUM bank)
KC2 = E // 256                 # 4 DoubleRow contraction pairs
NC_ = E // 128                 # 8 feature chunks
A_SCALE = 16.0                 # fp8 scale for the centered attn
W_SCALE = 32.0                 # host pre-scale of W1q/W2 (std 1/32 -> ~1)
OUT_DESCALE = 1.0 / (A_SCALE * 64.0 * W_SCALE)

_E4 = ml_dtypes.float8_e4m3
_F16 = np.float16


def build_nc(m_core=M_CORE) -> bass.Bass:
    n_stripes = m_core // MS
    nc = bacc.Bacc("TRN2", debug=False)

    xt = nc.dram_tensor("xt", [E, m_core], FP8, kind="ExternalInput")
    w1t = nc.dram_tensor("w1t", [E, E], FP8, kind="ExternalInput")
    w2t = nc.dram_tensor("w2t", [E, E], FP8, kind="ExternalInput")
    sel8 = nc.dram_tensor("sel8", [128, KC2 * 2 * 128], FP8, kind="ExternalInput")
    selt8 = nc.dram_tensor("selt8", [128, NC_ * 2 * 128], FP8, kind="ExternalInput")
    w2sum8 = nc.dram_tensor("w2sum8", [128, NC_ * 2 * 128], FP8, kind="ExternalInput")
    biasd = nc.dram_tensor("biasd", [128, NC_], F32, kind="ExternalInput")
    outT = nc.dram_tensor("outT", [E, m_core], F16, kind="ExternalOutput")

    xt_v = xt[:, :].rearrange("(c p) m -> p c m", p=128)    # [128, 8, m_core]
    w1_v = w1t[:, :].rearrange("(c p) n -> p c n", p=128)   # [128, 8, 1024]
    w2_v = w2t[:, :].rearrange("(c p) j -> p c j", p=128)   # [128, 8, 1024]

    with tile.TileContext(nc) as tc:
        with (
            tc.tile_pool(name="weights", bufs=1) as wpool,
            tc.tile_pool(name="xt", bufs=n_stripes) as xpool,
            tc.tile_pool(name="u", bufs=16) as upool,
            tc.tile_pool(name="u8", bufs=4) as u8pool,
            tc.tile_pool(name="at", bufs=8) as apool,
            tc.tile_pool(name="small", bufs=2) as spool,
            tc.tile_pool(name="ostage", bufs=8) as opool,
            tc.tile_pool(name="ps_q", bufs=3, space="PSUM") as psq,
            tc.tile_pool(name="ps_s", bufs=1, space="PSUM") as pss,
            tc.tile_pool(name="ps_rb", bufs=2, space="PSUM") as psrb,
            tc.tile_pool(name="ps_o", bufs=2, space="PSUM") as pso,
        ):
            # Warm the PE's HAM clock gate with small throwaway DR matmuls
            # (same mode as the real stream) while the first DMAs are in
            # flight.  gpsimd memset is ready right after its preamble
            # (~6.5us), before the PE's own preamble ends; N=128 keeps each
            # cold-clock warm matmul cheap so they don't delay stripe 0.
            warm_sb = wpool.tile([128, 2, 128], FP8, name="warm_sb")
            nc.gpsimd.memset(warm_sb[:], 0.0)
            warm_ps = psq.tile([128, MS], F32, tag="q", name="warm_ps")
            for _ in range(14):
                nc.tensor.matmul(
                    warm_ps[:, 0:128], warm_sb[:], warm_sb[:],
                    start=True, stop=True, perf_mode=DR,
                )

            # Weight/selector loads, ordered so stripe 0's mm1 can start
            # as early as possible.
            w1_k = []   # 4 tiles [128, 2, E] fp8 (DoubleRow pairs)
            x_k = [[None] * KC2 for _ in range(n_stripes)]
            # w1 on the sync HWDGE queue, stripe-0 x on the scalar-engine
            # HWDGE queue: both transfer in parallel so stripe 0's first
            # matmul isn't gated by a serial queue (ACT is idle here).
            for kc2 in range(KC2):
                t = wpool.tile([128, 2, E], FP8, name=f"w1k{kc2}")
                nc.sync.dma_start(t[:], w1_v[:, 2 * kc2:2 * kc2 + 2, :])
                w1_k.append(t)
                tx = xpool.tile([128, 2, MS], FP8, tag=f"xt_{kc2}", name=f"xt0_{kc2}")
                nc.scalar.dma_start(tx[:], xt_v[:, 2 * kc2:2 * kc2 + 2, 0:MS])
                x_k[0][kc2] = tx
            sel8_t = wpool.tile([128, KC2, 2, 128], FP8, name="sel8_t")
            nc.sync.dma_start(
                sel8_t[:], sel8[:, :].rearrange("p (c i g) -> p c i g", i=2, g=128)
            )
            selt8_t = wpool.tile([128, NC_, 2, 128], FP8, name="selt8_t")
            nc.sync.dma_start(
                selt8_t[:], selt8[:, :].rearrange("p (c i q) -> p c i q", i=2, q=128)
            )
            w2sum8_t = wpool.tile([128, NC_, 2, 128], FP8, name="w2sum8_t")
            nc.sync.dma_start(
                w2sum8_t[:], w2sum8[:, :].rearrange("p (c i q) -> p c i q", i=2, q=128)
            )
            bias_t = wpool.tile([128, NC_], F32, name="bias_t")
            nc.sync.dma_start(bias_t[:], biasd[:, :])

            w2_k = []   # 4 tiles [128, 2, E] fp8 (pairs of n-chunks)
            for c2 in range(KC2):
                t = wpool.tile([128, 2, E], FP8, name=f"w2k{c2}")
                nc.sync.dma_start(t[:], w2_v[:, 2 * c2:2 * c2 + 2, :])
                w2_k.append(t)

            def prefetch_x(ms):
                if ms < 1 or ms >= n_stripes:
                    return
                for kc2 in range(KC2):
                    tx = xpool.tile(
                        [128, 2, MS], FP8, tag=f"xt_{kc2}", name=f"xt{ms}_{kc2}"
                    )
                    nc.sync.dma_start(
                        tx[:], xt_v[:, 2 * kc2:2 * kc2 + 2, ms * MS:(ms + 1) * MS]
                    )
                    x_k[ms][kc2] = tx
            prefetch_x(1)

            # rcpc tiles [128, 2, MS] fp8 (K padded to 128 so every matmul
            # in the stream has the same (128,128) tile shape — K=16 tiles
            # measured +285ns/group): [0:16,0,:] = centered reciprocal
            # (DVE-written per stripe), [0,1,:] = A (the decenter constant,
            # written once), rest zero.  Two alternating buffers.
            rcpc_bufs = []
            for i in range(2):
                t = wpool.tile([128, 2, MS], FP8, name=f"rcpc{i}")
                nc.gpsimd.memset(t[:], 0.0)
                nc.gpsimd.memset(t[0:1, 1, :], A_SCALE)
                rcpc_bufs.append(t)

            def emit_rb_at0(prev, ci):
                """rb broadcast DR matmul + fused (u-1)*rb DVE op -> fp8."""
                rb_ps = psrb.tile([128, MS], F32, tag="rb", name="rb_ps")
                nc.tensor.matmul(
                    rb_ps[:], selt8_t[:, ci, :, :], prev["rcpc"][:],
                    start=True, stop=True, perf_mode=DR,
                )
                pair, half = divmod(ci, 2)
                nc.vector.scalar_tensor_tensor(
                    prev["at"][pair][:, half, :],
                    prev["u"][ci][:],
                    -1.0,
                    rb_ps[:],
                    op0=ALU.add,
                    op1=ALU.mult,
                )

            def emit_mm2(prev, js):
                """j-groups: [rcpc-corr DR (K=16), 4 fp8 DR at0 matmuls]."""
                for j in js:
                    o_ps = pso.tile([128, MS], F32, tag="o", name="o_ps")
                    nc.tensor.matmul(
                        o_ps[:],
                        w2sum8_t[:, j, :, :],
                        prev["rcpc"][:],
                        start=True,
                        stop=False,
                        perf_mode=DR,
                        skip_group_check=True,
                    )
                    for c2 in range(KC2):
                        nc.tensor.matmul(
                            o_ps[:],
                            w2_k[c2][:, :, j * 128:(j + 1) * 128],
                            prev["at"][c2][:],
                            start=False,
                            stop=(c2 == KC2 - 1),
                            perf_mode=DR,
                            skip_group_check=True,
                        )
                    o_t = opool.tile([128, MS], F16, tag="ost", name="o_t")
                    nc.scalar.activation(
                        o_t[:], o_ps[:], AF.Identity,
                        bias=bias_t[:, j:j + 1], scale=OUT_DESCALE,
                    )
                    nc.sync.dma_start(
                        outT[j * 128:(j + 1) * 128,
                             prev["ms"] * MS:(prev["ms"] + 1) * MS],
                        o_t[:],
                    )

            def emit_sel(cur):
                """4 DR head-sum matmuls on the fp8 u pairs + rcp chain."""
                s_ps = pss.tile([128, MS], F32, tag="s", name="s_ps")
                for c2 in range(KC2):
                    nc.tensor.matmul(
                        s_ps[:],
                        sel8_t[:, c2, :, :],
                        cur["u8"][c2][:],
                        start=(c2 == 0),
                        stop=(c2 == KC2 - 1),
                        perf_mode=DR,
                    )
                rcp32 = spool.tile([HEADS, MS], F32, tag="rcp32", name="rcp32")
                nc.vector.reciprocal_approx_fast(rcp32[:], s_ps[0:HEADS, :])
                nc.vector.tensor_scalar(
                    cur["rcpc"][0:HEADS, 0, :], rcp32[:], A_SCALE, A_SCALE,
                    op0=ALU.mult, op1=ALU.subtract,
                )

            prev = None
            for ms in range(n_stripes):
                prefetch_x(ms + 1)
                cur = {
                    "ms": ms,
                    "u": [],
                    "u8": [
                        u8pool.tile([128, 2, MS], FP8, tag=f"u8{p}", name=f"u8{ms}_{p}")
                        for p in range(KC2)
                    ],
                    "at": [
                        apool.tile([128, 2, MS], FP8, tag=f"at{p}", name=f"at{ms}_{p}")
                        for p in range(KC2)
                    ],
                    "rcpc": rcpc_bufs[ms % 2],
                }

                # Head of block: 2 rb matmuls of the previous stripe; the
                # rest interleave into the mm1 loop so the PE stays just
                # ahead of the DVE's 823ns/op at0 chain (psrb has 2 bufs).
                if prev is not None:
                    emit_rb_at0(prev, 0)
                    emit_rb_at0(prev, 1)

                for ci in range(NC_):
                    q_ps = psq.tile([128, MS], F32, tag="q", name="q_ps")
                    for kc2 in range(KC2):
                        nc.tensor.matmul(
                            q_ps[:],
                            w1_k[kc2][:, :, ci * 128:(ci + 1) * 128],
                            x_k[ms][kc2][:],
                            start=(kc2 == 0),
                            stop=(kc2 == KC2 - 1),
                            perf_mode=DR,
                        )
                    u_t = upool.tile([128, MS], F16, tag="u", name="u_t")
                    nc.scalar.activation(u_t[:], q_ps[:], AF.Exp, scale=1.0 / 128.0)
                    cur["u"].append(u_t)
                    if prev is not None and ci + 2 < NC_:
                        emit_rb_at0(prev, ci + 2)

                # u8 copies (DVE) emitted after all at0 ops so the at0
                # chain (needed first, by mm2) drains first.
                for ci in range(NC_):
                    nc.vector.tensor_copy(
                        cur["u8"][ci // 2][:, ci % 2, :], cur["u"][ci][:]
                    )

                last = ms == n_stripes - 1
                if prev is not None:
                    emit_mm2(prev, range(4 if last else 6))
                emit_sel(cur)
                if prev is None:
                    pass
                elif not last:
                    emit_mm2(prev, range(6, NC_))
                else:
                    # Final block: interleave the last stripe's rb/at0 chain
                    # into the remaining mm2 groups of the previous stripe so
                    # the epilogue's DVE work overlaps PE work.
                    for k, j in enumerate(range(4, NC_)):
                        emit_rb_at0(cur, k)
                        emit_mm2(prev, [j])
                    for ci in range(4, NC_):
                        emit_rb_at0(cur, ci)
                prev = cur

            # epilogue: last stripe's output projection (or, single-stripe
            # runs, its rb/at0 chain too)
            if n_stripes == 1:
                for ci in range(NC_):
                    emit_rb_at0(prev, ci)
            emit_mm2(prev, range(NC_))
    nc.compile()
    return nc


_NC_CACHE = None
LAST_RESULT = None


def _ensure_ntff_hook():
    """bass_utils' axon trace path needs antenv.axon_hooks, which this
    container's antenv lacks. Provide it + register the ctypes NTFF hook."""
    import types

    try:
        from antenv.axon_hooks import get_axon_ntff_profile_hook  # noqa: F401
        return True
    except ImportError:
        pass
    try:
        import antenv
        from trn_agent_boot.trn_boot import _ntff_profile_via_ctypes

        m = types.ModuleType("antenv.axon_hooks")
        state = {"hook": None}
        m.set_axon_ntff_profile_hook = lambda h: state.__setitem__("hook", h)
        m.get_axon_ntff_profile_hook = lambda: state["hook"]
        sys.modules["antenv.axon_hooks"] = m
        antenv.axon_hooks = m
        m.set_axon_ntff_profile_hook(
            _ntff_profile_via_ctypes("/opt/axon/libaxon_pjrt.so")
        )
        return True
    except Exception as e:  # pragma: no cover
        print(f"ntff hook injection failed: {e}")
        return False


def _selectors():
    # head index of global feature n is n // 64; chunk ci covers n in
    # [128ci, 128ci+128) -> heads 2ci (partitions 0..63), 2ci+1 (64..127)
    # All selector tiles are padded to the uniform (128, 2, 128) shape.
    sel8 = np.zeros((128, KC2, 2, 128), np.float32)
    for c2 in range(KC2):
        for i in range(2):
            ci = 2 * c2 + i
            sel8[:64, c2, i, 2 * ci] = 1.0 / 64.0
            sel8[64:, c2, i, 2 * ci + 1] = 1.0 / 64.0
    # selt8[:, ci, 0, :]: 0/1 head selector; [:, ci, 1, :]: row 0 = 1.0,
    # which multiplies the constant-A row of rcpc_pad -> rb = rcpc + A.
    selt8 = np.zeros((128, NC_, 2, 128), np.float32)
    for ci in range(NC_):
        selt8[2 * ci, ci, 0, :64] = 1.0
        selt8[2 * ci + 1, ci, 0, 64:] = 1.0
        selt8[0, ci, 1, :] = 1.0
    return (
        np.ascontiguousarray(sel8.reshape(128, KC2 * 2 * 128)).astype(_E4),
        np.ascontiguousarray(selt8.reshape(128, NC_ * 2 * 128)).astype(_E4),
    )


def _prep_weights(W1, W2):
    w1t = np.ascontiguousarray(W1[:E, :].T * W_SCALE).astype(_E4)  # [k, n]
    w2t = np.ascontiguousarray(W2.T * W_SCALE).astype(_E4)         # [n, j]
    # exact per-head column sums of 32*W2T (fp32)
    w2sum32 = (W2.T * W_SCALE).reshape(HEADS, HEAD_DIM, E).sum(axis=1)
    # w2sum8[0:16, j, 0, :] = fp8 per-head colsums (multiplies rcpc);
    # [:, j, 1, :] = 0 (the exact constant part goes through bias).
    w2sum8 = np.zeros((128, NC_, 2, 128), np.float32)
    for j in range(NC_):
        w2sum8[:HEADS, j, 0, :] = w2sum32[:, j * 128:(j + 1) * 128]
    bias = (w2sum32.sum(axis=0) * A_SCALE * OUT_DESCALE).astype(np.float32)
    biasd = np.ascontiguousarray(bias.reshape(NC_, 128).T)         # [128, NC_]
    return (
        w1t,
        w2t,
        np.ascontiguousarray(w2sum8.reshape(128, NC_ * 2 * 128)).astype(_E4),
        biasd,
    )


def kernel(x, W1, W2, heads, trace=False):
    global _NC_CACHE, LAST_RESULT
    x = np.asarray(x, dtype=np.float32)
    W1 = np.asarray(W1, dtype=np.float32)
    W2 = np.asarray(W2, dtype=np.float32)

    X = x.reshape(M_TOTAL, E)
    XT8 = np.ascontiguousarray(X.T).astype(_E4)  # [E, M_TOTAL]
    w1t, w2t, w2sum8, biasd = _prep_weights(W1, W2)
    sel8, selt8 = _selectors()

    in_maps = []
    for c in range(N_CORES):
        xt_c = np.ascontiguousarray(XT8[:, c * M_CORE:(c + 1) * M_CORE])
        in_maps.append(
            {"xt": xt_c, "w1t": w1t, "w2t": w2t, "sel8": sel8,
             "selt8": selt8, "w2sum8": w2sum8, "biasd": biasd}
        )

    if _NC_CACHE is None:
        _NC_CACHE = build_nc()

    if trace:
        trace = _ensure_ntff_hook()

    res = run_bass_kernel_spmd(_NC_CACHE, in_maps, list(range(N_CORES)), trace=trace)
    LAST_RESULT = res

    OT = np.concatenate(
        [np.asarray(res.results[c]["outT"]).astype(np.float32) for c in range(N_CORES)],
        axis=1,
    )
    return np.ascontiguousarray(OT.T).reshape(B, S, E)
